# revision 1
# baseline (speedup 1.0000x reference)
"""MPNN (LocalRetro) message-passing kernel for 8 Trainium2 NeuronCores.

Strategy (SPMD, one program, per-core input data):
- Edges sharded by dst-node range: core i owns edges whose dst lies in its
  1250-node slice, dst-sorted, packed into 128-slot chunks so that no dst
  node's edge group spans a chunk boundary. Pad slots yield e_hid == 0 via a
  -1e9 pad-indicator row appended to the edge-network input.
- msg = (e_hid (x) nf[src]) @ W2' computed without materializing edge_W:
  the Khatri-Rao product Z^T is built on DVE in bf16 via rotated partition
  windows of NF2 = [nf_src^T; nf_src^T] (no cross-partition broadcasts),
  then contracted on the PE against host-permuted W2 chunks (32 x K=128).
- Aggregation: a per-chunk one-hot matmul collapses each chunk's edges into
  per-dst-node rows (exact fp32); indirect DMA places the rows into the
  core's partial agg table (windows never overlap -> plain writes). A
  ReduceScatter sums partials across cores; each core runs the GRU for its
  1250 nodes in fp32 and AllGathers the new node state for the next step.
- Final sum-pooling via one-hot matmuls + AllReduce; core 0's output is
  returned.
"""
import sys

sys.path.insert(0, "/opt/trn_rl_repo")

import numpy as np
import ml_dtypes

from concourse import bass, bacc, mybir, tile, bass_utils
from concourse.masks import make_identity

BF16 = np.float16  # fp16: 10 mantissa bits, same PE/DVE rates as bf16

NCORES = 8
N_NODES = 10000
N_EDGES = 50000
N_GRAPHS = 128
NODE_IN = 74
EDGE_IN = 12
H = 64
STEPS = 6

N_LOC = N_NODES // NCORES            # 1250
N_PADLOC = 1280                      # local nodes padded to 10 tiles
NT_LOC = N_PADLOC // 128             # 10
NCHUNK = 56                          # 128-slot edge chunks per core
E_PAD = NCHUNK * 128                 # 7168 edge slots
NTILE = E_PAD // 512                 # 14 msg tiles of 512
TRASH = N_NODES                      # agg rows >= N_NODES are scratch
AGG_ROWS = 10240                     # zeroed region (>= TRASH + 128)
GRU_TILES = [(0, 512), (512, 512), (1024, 256)]
ACT_RELU = mybir.ActivationFunctionType.Relu
ACT_SIG = mybir.ActivationFunctionType.Sigmoid
ACT_TANH = mybir.ActivationFunctionType.Tanh
ACT_COPY = mybir.ActivationFunctionType.Copy


def prep_host(inp):
    """Build per-core and shared device input arrays from the full inputs."""
    W_e2 = inp["W_e2"]
    # W2 chunks permuted for the rotated Khatri-Rao layout:
    # chunk j, row i      -> (c=i, h=(2j+i)%64)
    # chunk j, row 64+i   -> (c=i, h=(2j+1+i)%64)
    W2r = W_e2.reshape(H, H, H)
    W2P = np.zeros((32, 128, H), np.float32)
    i = np.arange(64)
    for j in range(32):
        W2P[j, :64, :] = W2r[2 * j, i, :]        # (c=2j,   h=i)
        W2P[j, 64:, :] = W2r[2 * j + 1, i, :]    # (c=2j+1, h=i)
    SEL = np.zeros((32, H, 128), np.float32)
    for j in range(32):
        SEL[j, 2 * j, :64] = 1.0
        SEL[j, 2 * j + 1, 64:] = 1.0
    b_ih, b_hh = inp["b_ih"], inp["b_hh"]
    b_rz = b_ih + b_hh
    bias_cols = np.stack([
        b_rz[0:64], b_rz[64:128],          # r, z sigmoid biases
        b_ih[128:192], b_hh[128:192],      # n-gate: i_n bias, h_n bias
        inp["conv_bias"], inp["b_proj"], inp["b_e1"],
    ], axis=1).astype(np.float32)          # [64, 7]

    shared = {
        "W_proj": inp["W_proj"].astype(np.float32),
        "W1a": np.concatenate([inp["W_e1"],
                               np.full((1, H), -1e9, np.float32)], 0),
        "W2P": W2P.astype(BF16),
        "SEL": SEL.astype(BF16),
        "B2": inp["b_e2"].reshape(H, H).astype(BF16),
        "WihT": np.ascontiguousarray(inp["W_ih"].T).astype(np.float32),
        "WhhT": np.ascontiguousarray(inp["W_hh"].T).astype(np.float32),
        "bias_cols": bias_cols,
    }

    src, dst = np.asarray(inp["src"]), np.asarray(inp["dst"])
    ef, gids = np.asarray(inp["edge_feats"]), np.asarray(inp["graph_ids"])
    nft = np.ascontiguousarray(np.asarray(inp["node_feats"]).T).astype(np.float32)
    order = np.argsort(dst, kind="stable")
    dst_s, src_s, ef_s = dst[order], src[order], ef[order]

    cores = []
    for ci in range(NCORES):
        lo, hi = ci * N_LOC, (ci + 1) * N_LOC
        sel = (dst_s >= lo) & (dst_s < hi)
        d, s, e = dst_s[sel], src_s[sel], ef_s[sel]
        ne = len(d)
        starts = np.flatnonzero(np.concatenate([[True], d[1:] != d[:-1]])) if ne else np.array([], np.int64)
        ends = np.concatenate([starts[1:], [ne]]) if ne else np.array([], np.int64)
        slot_src = np.zeros(E_PAD, np.int32)
        slot_pad = np.ones(E_PAD, np.float32)
        slot_ef = np.zeros((E_PAD, EDGE_IN), np.float32)
        S = np.zeros((NCHUNK, 128, 128), np.float32)
        scat_idx = np.tile(np.arange(TRASH, TRASH + 128, dtype=np.int32)[:, None],
                           (1, NCHUNK))
        chunk, pos, slot = 0, 0, 0
        for g in range(len(starts)):
            glen = int(ends[g] - starts[g])
            assert glen <= 128, "node degree exceeds one chunk"
            if pos + glen > 128:
                chunk, pos, slot = chunk + 1, 0, 0
            assert chunk < NCHUNK, "NCHUNK too small"
            b = chunk * 128
            sl = slice(int(starts[g]), int(ends[g]))
            slot_src[b + pos:b + pos + glen] = s[sl]
            slot_pad[b + pos:b + pos + glen] = 0.0
            slot_ef[b + pos:b + pos + glen] = e[sl]
            S[chunk, pos:pos + glen, slot] = 1.0
            scat_idx[slot, chunk] = int(d[int(starts[g])])
            pos += glen
            slot += 1
        eft = np.zeros((EDGE_IN + 1, E_PAD), np.float32)
        eft[:EDGE_IN] = slot_ef.T
        eft[EDGE_IN] = slot_pad
        g_loc = gids[lo:hi]
        SP = np.zeros((NT_LOC, 128, N_GRAPHS), np.float32)
        for t in range(NT_LOC):
            cnt = min(128, N_LOC - t * 128)
            SP[t, np.arange(cnt), g_loc[t * 128:t * 128 + cnt]] = 1.0
        nft_loc = np.zeros((NODE_IN, N_PADLOC), np.float32)
        nft_loc[:, :N_LOC] = nft[:, lo:hi]
        core = dict(shared)
        core.update({
            "eft": eft,
            "gsrc": np.ascontiguousarray(slot_src.reshape(NCHUNK, 128).T).astype(np.int32),
            "scat_idx": scat_idx.astype(np.int32),
            "S": S.astype(BF16), "SP": SP.astype(BF16), "nft_loc": nft_loc,
        })
        cores.append(core)
    return cores


def build_bass():
    nc = bacc.Bacc("TRN2", target_bir_lowering=False, debug=False,
                   num_devices=NCORES)
    dt = mybir.dt
    f32, bf16, i32 = dt.float32, dt.float16, dt.int32

    def din(name, shape, dtype):
        return nc.dram_tensor(name, shape, dtype, kind="ExternalInput")

    nft_d = din("nft_loc", [NODE_IN, N_PADLOC], f32)
    Wp_d = din("W_proj", [NODE_IN, H], f32)
    W1a_d = din("W1a", [EDGE_IN + 1, H], f32)
    W2P_d = din("W2P", [32, 128, H], bf16)
    SEL_d = din("SEL", [32, H, 128], bf16)
    B2_d = din("B2", [H, H], bf16)
    WihT_d = din("WihT", [H, 3 * H], f32)
    WhhT_d = din("WhhT", [H, 3 * H], f32)
    bias_d = din("bias_cols", [H, 7], f32)
    eft_d = din("eft", [EDGE_IN + 1, E_PAD], f32)
    gsrc_d = din("gsrc", [128, NCHUNK], i32)
    scat_d = din("scat_idx", [128, NCHUNK], i32)
    S_d = din("S", [NCHUNK, 128, 128], bf16)
    SP_d = din("SP", [NT_LOC, 128, N_GRAPHS], bf16)
    out_d = nc.dram_tensor("g_feat", [N_GRAPHS, H], f32, kind="ExternalOutput")

    RG = [list(range(NCORES))]

    with tile.TileContext(nc) as tc:
        with tc.tile_pool(name="const", bufs=1) as cpool, \
             tc.tile_pool(name="state", bufs=1) as spool, \
             tc.tile_pool(name="psM", bufs=2, space="PSUM") as psM, \
             tc.tile_pool(name="psB", bufs=2, space="PSUM") as psB, \
             tc.tile_pool(name="psT", bufs=2, space="PSUM") as psT, \
             tc.tile_pool(name="psG", bufs=2, space="PSUM") as psG, \
             tc.tile_pool(name="dram", bufs=1, space="DRAM") as dpool:

            ident = cpool.tile([128, 128], f32)
            make_identity(nc, ident[:])
            ident16 = cpool.tile([128, 128], bf16)
            make_identity(nc, ident16[:])

            # ---- constants to SBUF ----
            Wp_sb = cpool.tile([NODE_IN, H], f32)
            nc.sync.dma_start(Wp_sb[:], Wp_d[:])
            W1a_sb = cpool.tile([EDGE_IN + 1, H], f32)
            nc.sync.dma_start(W1a_sb[:], W1a_d[:])
            W2P_sb = cpool.tile([128, 32 * H], bf16)
            for j in range(32):
                nc.sync.dma_start(W2P_sb[:, j * H:(j + 1) * H], W2P_d[j])
            SEL_sb = cpool.tile([H, 32 * 128], bf16)
            for j in range(32):
                nc.sync.dma_start(SEL_sb[:, j * 128:(j + 1) * 128], SEL_d[j])
            B2_sb = cpool.tile([H, H], bf16)
            nc.sync.dma_start(B2_sb[:], B2_d[:])
            Wih_sb = cpool.tile([H, 3 * H], f32)
            nc.sync.dma_start(Wih_sb[:], WihT_d[:])
            Whh_sb = cpool.tile([H, 3 * H], f32)
            nc.sync.dma_start(Whh_sb[:], WhhT_d[:])
            bias_sb = cpool.tile([H, 7], f32)
            nc.sync.dma_start(bias_sb[:], bias_d[:])
            gsrc_sb = cpool.tile([128, NCHUNK], i32)
            nc.sync.dma_start(gsrc_sb[:], gsrc_d[:])
            scat_sb = cpool.tile([128, NCHUNK], i32)
            nc.sync.dma_start(scat_sb[:], scat_d[:])
            S_sb = cpool.tile([128, NCHUNK * 128], bf16)
            for c in range(NCHUNK):
                nc.sync.dma_start(S_sb[:, c * 128:(c + 1) * 128], S_d[c])
            SP_sb = cpool.tile([128, NT_LOC * N_GRAPHS], bf16)
            for t in range(NT_LOC):
                nc.sync.dma_start(SP_sb[:, t * N_GRAPHS:(t + 1) * N_GRAPHS], SP_d[t])
            zeros = cpool.tile([128, 640], f32)
            nc.vector.memset(zeros[:], 0.0)

            # ---- DRAM scratch ----
            nf_loc = dpool.tile([N_LOC, H], bf16)
            nf_fulls = [dpool.tile([N_NODES, H], bf16, addr_space="Shared", name=f"nf_full{i}") for i in range(STEPS)]
            agg_part = dpool.tile([AGG_ROWS, H], f32)
            agg_loc = dpool.tile([N_LOC, H], f32)
            pool_part = dpool.tile([N_GRAPHS, H], f32)
            pool_out = dpool.tile([N_GRAPHS, H], f32, addr_space="Shared")

            # ---- persistent state ----
            EH1 = spool.tile([H, E_PAD], bf16)
            NF2 = spool.tile([128, E_PAD], bf16)
            gbuf = spool.tile([128, NCHUNK * H], bf16)
            msg_rows = spool.tile([128, NCHUNK * H], bf16)
            scat_rows = spool.tile([128, NCHUNK * H], f32)
            nfrow = spool.tile([128, NT_LOC * H], bf16)
            hidA = spool.tile([H, N_PADLOC], f32)
            hidB = spool.tile([H, N_PADLOC], f32)
            xT = spool.tile([H, N_PADLOC], f32)
            nc.vector.memset(xT[:], 0.0)

            # ========== prep phase (transient inputs) ==========
            with tc.tile_pool(name="prep", bufs=1) as ppool:
                eft_sb = ppool.tile([EDGE_IN + 1, E_PAD], f32)
                nc.sync.dma_start(eft_sb[:], eft_d[:])
                nftl_sb = ppool.tile([NODE_IN, N_PADLOC], f32)
                nc.sync.dma_start(nftl_sb[:], nft_d[:])
                # edge hidden, doubled (once)
                for ti in range(NTILE):
                    sl = slice(ti * 512, (ti + 1) * 512)
                    ps = psM.tile([H, 512], f32, tag="m")
                    nc.tensor.matmul(ps[:], lhsT=W1a_sb[:], rhs=eft_sb[:, sl],
                                     start=True, stop=True)
                    nc.scalar.activation(EH1[:, sl], ps[:], ACT_RELU,
                                         bias=bias_sb[:, 6:7])
                # initial node state (once)
                for t in range(NT_LOC):
                    sl = slice(t * 128, (t + 1) * 128)
                    ps = psM.tile([H, 128], f32, tag="m")
                    nc.tensor.matmul(ps[:], lhsT=Wp_sb[:], rhs=nftl_sb[:, sl],
                                     start=True, stop=True)
                    nc.scalar.activation(hidA[:, sl], ps[:], ACT_RELU,
                                         bias=bias_sb[:, 5:6])

            wstack = tc.tile_pool(name="work", bufs=2)
            wpool = wstack.__enter__()
            zstack = tc.tile_pool(name="zt", bufs=2)
            zpool = zstack.__enter__()

            def write_state_rows(hid):
                """hid^T [64, N_PADLOC] -> nfrow row tiles -> nf_loc (DRAM)."""
                for t in range(NT_LOC):
                    pt = psT.tile([128, H], f32, tag="t")
                    nc.tensor.transpose(pt[:], in_=hid[:, t * 128:(t + 1) * 128],
                                        identity=ident[:H, :H])
                    nc.scalar.activation(nfrow[:, t * H:(t + 1) * H], pt[:], ACT_COPY)
                for t in range(NT_LOC):
                    cnt = min(128, N_LOC - t * 128)
                    nc.sync.dma_start(nf_loc[t * 128:t * 128 + cnt, :],
                                      nfrow[:cnt, t * H:(t + 1) * H])

            write_state_rows(hidA)

            hid_cur, hid_nxt = hidA, hidB
            for step in range(STEPS):
                # ---- 1. AllGather node state rows ----
                nf_full = nf_fulls[step]
                nc.gpsimd.collective_compute(
                    "AllGather", mybir.AluOpType.bypass, replica_groups=RG,
                    ins=[nf_loc[:]], outs=[nf_full[:]])

                # ---- 2. gather nf[src] rows ----
                for c in range(NCHUNK):
                    nc.gpsimd.indirect_dma_start(
                        out=gbuf[:, c * H:(c + 1) * H],
                        out_offset=None,
                        in_=nf_full[:, :],
                        in_offset=bass.IndirectOffsetOnAxis(
                            ap=gsrc_sb[:, c:c + 1], axis=0))

                # ---- 3. transpose into NF2 (bf16, doubled) ----
                for c in range(NCHUNK):
                    pt = psT.tile([H, 128], bf16, tag="t")
                    nc.tensor.transpose(pt[:], in_=gbuf[:, c * H:(c + 1) * H],
                                        identity=ident16[:, :])
                    sl = slice(c * 128, (c + 1) * 128)
                    nc.scalar.activation(NF2[0:H, sl], pt[:], ACT_COPY)
                nc.sync.dma_start(NF2[H:128, :], NF2[0:H, :])

                # ---- 4. msg^T = W2P-chunks @ Z^T ----
                # Z chunk j row p: (c = 2j + (p>=64), h = p%64)
                # = (SEL[j]-bcast of e_hid rows) * NF2
                for ti in range(NTILE):
                    sl = slice(ti * 512, (ti + 1) * 512)
                    pm = psM.tile([H, 512], f32, tag="m")
                    for j in range(32):
                        pbc = psB.tile([128, 512], f32, tag="b")
                        nc.tensor.matmul(pbc[:], lhsT=SEL_sb[:, j * 128:(j + 1) * 128],
                                         rhs=EH1[:, sl], start=True, stop=True)
                        ebc = wpool.tile([128, 512], bf16, tag="ebc")
                        nc.scalar.activation(ebc[:], pbc[:], ACT_COPY)
                        zt = zpool.tile([128, 512], bf16, tag="zt")
                        nc.vector.tensor_mul(zt[:], ebc[:], NF2[:, sl])
                        nc.tensor.matmul(pm[:], lhsT=W2P_sb[:, j * H:(j + 1) * H],
                                         rhs=zt[:], start=(j == 0), stop=False)
                    nc.tensor.matmul(pm[:], lhsT=B2_sb[:], rhs=NF2[0:H, sl],
                                     start=False, stop=True)
                    # transpose msg^T tile back to row layout (via SBUF)
                    mT = wpool.tile([H, 512], f32, tag="mT")
                    nc.scalar.activation(mT[:], pm[:], ACT_COPY)
                    for h4 in range(4):
                        cb = ti * 4 + h4
                        pt = psT.tile([128, H], f32, tag="t")
                        nc.tensor.transpose(
                            pt[:], in_=mT[:, h4 * 128:(h4 + 1) * 128],
                            identity=ident[:H, :H])
                        nc.scalar.activation(
                            msg_rows[:, cb * H:(cb + 1) * H], pt[:], ACT_COPY)

                # ---- 5. per-chunk dedup matmul + indirect scatter ----
                for a in range(AGG_ROWS // 1280):
                    nc.sync.dma_start(
                        agg_part[a * 1280:(a + 1) * 1280, :].rearrange(
                            "(p r) h -> p (r h)", p=128),
                        zeros[:])
                for c in range(NCHUNK):
                    ps = psT.tile([128, H], f32, tag="t")
                    nc.tensor.matmul(
                        ps[:], lhsT=S_sb[:, c * 128:(c + 1) * 128],
                        rhs=msg_rows[:, c * H:(c + 1) * H], start=True, stop=True)
                    nc.scalar.activation(scat_rows[:, c * H:(c + 1) * H], ps[:],
                                         ACT_COPY)
                for c in range(NCHUNK):
                    nc.gpsimd.indirect_dma_start(
                        out=agg_part[:, :],
                        out_offset=bass.IndirectOffsetOnAxis(
                            ap=scat_sb[:, c:c + 1], axis=0),
                        in_=scat_rows[:, c * H:(c + 1) * H],
                        in_offset=None)

                # ---- 6. ReduceScatter partial aggs ----
                nc.gpsimd.collective_compute(
                    "ReduceScatter", mybir.AluOpType.add, replica_groups=RG,
                    ins=[agg_part[:N_NODES, :]], outs=[agg_loc[:]])

                # ---- 7. x^T = relu(agg + conv_bias) ----
                for t in range(NT_LOC):
                    cnt = min(128, N_LOC - t * 128)
                    xr = wpool.tile([128, H], f32, tag="xrow")
                    nc.sync.dma_start(xr[:cnt, :], agg_loc[t * 128:t * 128 + cnt, :])
                    pt = psT.tile([H, 128], f32, tag="t")
                    nc.tensor.transpose(pt[:, :cnt], in_=xr[:cnt, :],
                                        identity=ident[:cnt, :cnt])
                    nc.scalar.activation(xT[:, t * 128:t * 128 + cnt],
                                         pt[:, :cnt], ACT_RELU,
                                         bias=bias_sb[:, 4:5])

                # ---- 8. GRU (local nodes, fp32) ----
                for (n0, nw) in GRU_TILES:
                    xsl = xT[:, n0:n0 + nw]
                    hsl = hid_cur[:, n0:n0 + nw]
                    # r gate
                    pr = psG.tile([H, 512], f32, tag="gate")
                    nc.tensor.matmul(pr[:, :nw], lhsT=Wih_sb[:, 0:H], rhs=xsl,
                                     start=True, stop=False)
                    nc.tensor.matmul(pr[:, :nw], lhsT=Whh_sb[:, 0:H], rhs=hsl,
                                     start=False, stop=True)
                    r = wpool.tile([H, 512], f32, tag="gr", bufs=1)
                    nc.scalar.activation(r[:, :nw], pr[:, :nw], ACT_SIG,
                                         bias=bias_sb[:, 0:1])
                    # z gate
                    pz = psG.tile([H, 512], f32, tag="gate")
                    nc.tensor.matmul(pz[:, :nw], lhsT=Wih_sb[:, H:2 * H], rhs=xsl,
                                     start=True, stop=False)
                    nc.tensor.matmul(pz[:, :nw], lhsT=Whh_sb[:, H:2 * H], rhs=hsl,
                                     start=False, stop=True)
                    z = wpool.tile([H, 512], f32, tag="gz", bufs=1)
                    nc.scalar.activation(z[:, :nw], pz[:, :nw], ACT_SIG,
                                         bias=bias_sb[:, 1:2])
                    # n gate: n = tanh(i_n + b_in + r*(h_n + b_hn))
                    phn = psG.tile([H, 512], f32, tag="gate")
                    nc.tensor.matmul(phn[:, :nw], lhsT=Whh_sb[:, 2 * H:3 * H],
                                     rhs=hsl, start=True, stop=True)
                    hn = wpool.tile([H, 512], f32, tag="ghn", bufs=1)
                    nc.scalar.activation(hn[:, :nw], phn[:, :nw],
                                         mybir.ActivationFunctionType.Identity,
                                         bias=bias_sb[:, 3:4])
                    pin = psG.tile([H, 512], f32, tag="gate")
                    nc.tensor.matmul(pin[:, :nw], lhsT=Wih_sb[:, 2 * H:3 * H],
                                     rhs=xsl, start=True, stop=True)
                    rn = wpool.tile([H, 512], f32, tag="grn", bufs=1)
                    nc.vector.tensor_mul(rn[:, :nw], r[:, :nw], hn[:, :nw])
                    tmp = wpool.tile([H, 512], f32, tag="gtmp", bufs=1)
                    nc.vector.tensor_add(tmp[:, :nw], rn[:, :nw], pin[:, :nw])
                    n_g = wpool.tile([H, 512], f32, tag="gn", bufs=1)
                    nc.scalar.activation(n_g[:, :nw], tmp[:, :nw], ACT_TANH,
                                         bias=bias_sb[:, 2:3])
                    # h' = n + z*(h - n)
                    hmn = wpool.tile([H, 512], f32, tag="ghmn", bufs=1)
                    nc.vector.tensor_sub(hmn[:, :nw], hsl, n_g[:, :nw])
                    zh = wpool.tile([H, 512], f32, tag="gzh", bufs=1)
                    nc.vector.tensor_mul(zh[:, :nw], z[:, :nw], hmn[:, :nw])
                    nc.vector.tensor_add(hid_nxt[:, n0:n0 + nw], n_g[:, :nw],
                                         zh[:, :nw])

                # ---- 9. write new state rows (skip DMA on last step) ----
                if step < STEPS - 1:
                    write_state_rows(hid_nxt)
                else:
                    for t in range(NT_LOC):
                        pt = psT.tile([128, H], f32, tag="t")
                        nc.tensor.transpose(
                            pt[:], in_=hid_nxt[:, t * 128:(t + 1) * 128],
                            identity=ident[:H, :H])
                        nc.scalar.activation(nfrow[:, t * H:(t + 1) * H], pt[:],
                                             ACT_COPY)
                hid_cur, hid_nxt = hid_nxt, hid_cur

            # ========== sum pooling + AllReduce ==========
            pp = psM.tile([N_GRAPHS, H], f32, tag="m")
            for t in range(NT_LOC):
                nc.tensor.matmul(pp[:], lhsT=SP_sb[:, t * N_GRAPHS:(t + 1) * N_GRAPHS],
                                 rhs=nfrow[:, t * H:(t + 1) * H],
                                 start=(t == 0), stop=(t == NT_LOC - 1))
            pool_sb = wpool.tile([N_GRAPHS, H], f32, tag="pool")
            nc.scalar.activation(pool_sb[:], pp[:], ACT_COPY)
            nc.sync.dma_start(pool_part[:], pool_sb[:])
            nc.gpsimd.collective_compute(
                "AllReduce", mybir.AluOpType.add, replica_groups=RG,
                ins=[pool_part[:]], outs=[pool_out[:]])
            nc.sync.dma_start(out_d[:], pool_out[:])
            zstack.__exit__(None, None, None)
            wstack.__exit__(None, None, None)

    nc.compile()
    return nc


_CACHED = {}


def _get_nc():
    if "nc" not in _CACHED:
        _CACHED["nc"] = build_bass()
    return _CACHED["nc"]


def kernel(**inputs):
    np_inputs = {k: np.asarray(v) for k, v in inputs.items()}
    in_maps = prep_host(np_inputs)
    nc = _get_nc()
    res = bass_utils.run_bass_kernel_spmd(
        nc, in_maps, core_ids=list(range(NCORES)))
    return res.results[0]["g_feat"]


if __name__ == "__main__":
    # quick numpy self-check of the host-side math on random data
    rng = np.random.default_rng(0)
    print("kernel module OK")



# revision 5
# speedup vs baseline: 1.5591x; 1.5591x over previous
"""MPNN (LocalRetro) message-passing kernel for 8 Trainium2 NeuronCores.

Strategy (SPMD, one program, per-core input data):
- Edges sharded by SRC-node range: core i owns edges whose src lies in its
  1250-node slice, dst-sorted within the core, packed into 128-slot chunks
  so that no dst group spans a chunk boundary. The nf[src] gather is then
  core-LOCAL (no AllGather). Pad slots yield e_hid == 0 via a -1e9
  pad-indicator row appended to the edge-network input.
- msg = (e_hid (x) nf[src]) @ W2' computed without materializing edge_W:
  the Khatri-Rao product Z^T is built on DVE in fp16 as EBC_j * NF2 where
  EBC_j (the j-th c-pair of e_hid rows broadcast across partitions) is
  step-invariant and precomputed once into DRAM via broadcast DMA, then
  streamed per step. Contraction on the PE against host-permuted W2 chunks
  (32 x K=128) in wide 2048-column tiles.
- Aggregation: a per-chunk one-hot matmul collapses each chunk's edges into
  per-dst-node rows (exact fp32); indirect DMA places the rows into the
  core's partial agg table over ALL nodes (rows never collide within a
  core). A ReduceScatter sums partials across cores; each core runs the
  GRU for its 1250 nodes in fp32.
- Final sum-pooling via one-hot matmuls + AllReduce; core 0's output is
  returned.
"""
import sys

sys.path.insert(0, "/opt/trn_rl_repo")

import numpy as np
import ml_dtypes

from concourse import bass, bacc, mybir, tile, bass_utils
from concourse.masks import make_identity

BF16 = np.float16  # fp16: 10 mantissa bits, same PE/DVE rates as bf16

NCORES = 8
N_NODES = 10000
N_EDGES = 50000
N_GRAPHS = 128
NODE_IN = 74
EDGE_IN = 12
H = 64
STEPS = 6

N_LOC = N_NODES // NCORES            # 1250
N_PADLOC = 1280                      # local nodes padded to 10 tiles
NT_LOC = N_PADLOC // 128             # 10
NCHUNK = 56                          # 128-slot edge chunks per core
E_PAD = NCHUNK * 128                 # 7168 edge slots
TILES = [(0, 2048), (2048, 2048), (4096, 2048), (6144, 1024)]
TW_MAX = 2048
TRASH = N_NODES                      # agg rows >= N_NODES are scratch
AGG_ROWS = 10240                     # zeroed region (>= TRASH + 128)
GRU_TILES = [(0, 512), (512, 512), (1024, 256)]
ACT_RELU = mybir.ActivationFunctionType.Relu
ACT_SIG = mybir.ActivationFunctionType.Sigmoid
ACT_TANH = mybir.ActivationFunctionType.Tanh
ACT_COPY = mybir.ActivationFunctionType.Copy


def prep_host(inp):
    """Build per-core and shared device input arrays from the full inputs."""
    W_e2 = inp["W_e2"]
    # W2 chunks for the c-pair Khatri-Rao layout:
    # chunk j, row i      -> (c=2j,   h=i)
    # chunk j, row 64+i   -> (c=2j+1, h=i)
    W2r = W_e2.reshape(H, H, H)
    W2P = np.zeros((32, 128, H), np.float32)
    i = np.arange(64)
    for j in range(32):
        W2P[j, :64, :] = W2r[2 * j, i, :]
        W2P[j, 64:, :] = W2r[2 * j + 1, i, :]
    b_ih, b_hh = inp["b_ih"], inp["b_hh"]
    b_rz = b_ih + b_hh
    bias_cols = np.stack([
        b_rz[0:64], b_rz[64:128],          # r, z sigmoid biases
        b_ih[128:192], b_hh[128:192],      # n-gate: i_n bias, h_n bias
        inp["conv_bias"], inp["b_proj"], inp["b_e1"],
    ], axis=1).astype(np.float32)          # [64, 7]

    shared = {
        "W_proj": inp["W_proj"].astype(np.float32),
        "W1a": np.concatenate([inp["W_e1"],
                               np.full((1, H), -1e9, np.float32)], 0),
        "W2P": W2P.astype(BF16),
        "B2": inp["b_e2"].reshape(H, H).astype(BF16),
        "WihT": np.ascontiguousarray(inp["W_ih"].T).astype(np.float32),
        "WhhT": np.ascontiguousarray(inp["W_hh"].T).astype(np.float32),
        "bias_cols": bias_cols,
    }

    src, dst = np.asarray(inp["src"]), np.asarray(inp["dst"])
    ef, gids = np.asarray(inp["edge_feats"]), np.asarray(inp["graph_ids"])
    nft = np.ascontiguousarray(np.asarray(inp["node_feats"]).T).astype(np.float32)
    order = np.argsort(dst, kind="stable")
    dst_s, src_s, ef_s = dst[order], src[order], ef[order]

    cores = []
    for ci in range(NCORES):
        lo, hi = ci * N_LOC, (ci + 1) * N_LOC
        sel = (src_s >= lo) & (src_s < hi)       # shard by SRC range
        d, s, e = dst_s[sel], src_s[sel], ef_s[sel]
        ne = len(d)
        assert ne <= E_PAD, f"core {ci}: {ne} edges > {E_PAD}"
        starts = np.flatnonzero(np.concatenate([[True], d[1:] != d[:-1]])) if ne else np.array([], np.int64)
        ends = np.concatenate([starts[1:], [ne]]) if ne else np.array([], np.int64)
        slot_src = np.zeros(E_PAD, np.int32)     # LOCAL src row index
        slot_pad = np.ones(E_PAD, np.float32)
        slot_ef = np.zeros((E_PAD, EDGE_IN), np.float32)
        S = np.zeros((NCHUNK, 128, 128), np.float32)
        scat_idx = np.tile(np.arange(TRASH, TRASH + 128, dtype=np.int32)[:, None],
                           (1, NCHUNK))
        chunk, pos, slot = 0, 0, 0
        for g in range(len(starts)):
            glen = int(ends[g] - starts[g])
            assert glen <= 128, "node degree exceeds one chunk"
            if pos + glen > 128:
                chunk, pos, slot = chunk + 1, 0, 0
            assert chunk < NCHUNK, "NCHUNK too small"
            b = chunk * 128
            sl = slice(int(starts[g]), int(ends[g]))
            slot_src[b + pos:b + pos + glen] = s[sl] - lo
            slot_pad[b + pos:b + pos + glen] = 0.0
            slot_ef[b + pos:b + pos + glen] = e[sl]
            S[chunk, pos:pos + glen, slot] = 1.0
            scat_idx[slot, chunk] = int(d[int(starts[g])])
            pos += glen
            slot += 1
        eft = np.zeros((EDGE_IN + 1, E_PAD), np.float32)
        eft[:EDGE_IN] = slot_ef.T
        eft[EDGE_IN] = slot_pad
        g_loc = gids[lo:hi]
        SP = np.zeros((NT_LOC, 128, N_GRAPHS), np.float32)
        for t in range(NT_LOC):
            cnt = min(128, N_LOC - t * 128)
            SP[t, np.arange(cnt), g_loc[t * 128:t * 128 + cnt]] = 1.0
        nft_loc = np.zeros((NODE_IN, N_PADLOC), np.float32)
        nft_loc[:, :N_LOC] = nft[:, lo:hi]
        core = dict(shared)
        core.update({
            "eft": eft,
            "gsrc": np.ascontiguousarray(slot_src.reshape(NCHUNK, 128).T).astype(np.int32),
            "scat_idx": scat_idx.astype(np.int32),
            "S": S.astype(BF16), "SP": SP.astype(BF16), "nft_loc": nft_loc,
        })
        cores.append(core)
    return cores


def build_bass():
    nc = bacc.Bacc("TRN2", target_bir_lowering=False, debug=False,
                   num_devices=NCORES)
    dt = mybir.dt
    f32, bf16, i32 = dt.float32, dt.float16, dt.int32

    def din(name, shape, dtype):
        return nc.dram_tensor(name, shape, dtype, kind="ExternalInput")

    nft_d = din("nft_loc", [NODE_IN, N_PADLOC], f32)
    Wp_d = din("W_proj", [NODE_IN, H], f32)
    W1a_d = din("W1a", [EDGE_IN + 1, H], f32)
    W2P_d = din("W2P", [32, 128, H], bf16)
    B2_d = din("B2", [H, H], bf16)
    WihT_d = din("WihT", [H, 3 * H], f32)
    WhhT_d = din("WhhT", [H, 3 * H], f32)
    bias_d = din("bias_cols", [H, 7], f32)
    eft_d = din("eft", [EDGE_IN + 1, E_PAD], f32)
    gsrc_d = din("gsrc", [128, NCHUNK], i32)
    scat_d = din("scat_idx", [128, NCHUNK], i32)
    S_d = din("S", [NCHUNK, 128, 128], bf16)
    SP_d = din("SP", [NT_LOC, 128, N_GRAPHS], bf16)
    out_d = nc.dram_tensor("g_feat", [N_GRAPHS, H], f32, kind="ExternalOutput")

    RG = [list(range(NCORES))]

    with tile.TileContext(nc) as tc:
        with tc.tile_pool(name="const", bufs=1) as cpool, \
             tc.tile_pool(name="state", bufs=1) as spool, \
             tc.tile_pool(name="psM", bufs=1, space="PSUM") as psM, \
             tc.tile_pool(name="psT", bufs=2, space="PSUM") as psT, \
             tc.tile_pool(name="psG", bufs=2, space="PSUM") as psG, \
             tc.tile_pool(name="dram", bufs=1, space="DRAM") as dpool:

            ident = cpool.tile([128, 128], f32)
            make_identity(nc, ident[:])
            ident16 = cpool.tile([128, 128], bf16)
            make_identity(nc, ident16[:])

            # ---- constants to SBUF ----
            Wp_sb = cpool.tile([NODE_IN, H], f32)
            nc.sync.dma_start(Wp_sb[:], Wp_d[:])
            W1a_sb = cpool.tile([EDGE_IN + 1, H], f32)
            nc.sync.dma_start(W1a_sb[:], W1a_d[:])
            W2P_sb = cpool.tile([128, 32 * H], bf16)
            for j in range(32):
                nc.sync.dma_start(W2P_sb[:, j * H:(j + 1) * H], W2P_d[j])
            B2_sb = cpool.tile([H, H], bf16)
            nc.sync.dma_start(B2_sb[:], B2_d[:])
            Wih_sb = cpool.tile([H, 3 * H], f32)
            nc.sync.dma_start(Wih_sb[:], WihT_d[:])
            Whh_sb = cpool.tile([H, 3 * H], f32)
            nc.sync.dma_start(Whh_sb[:], WhhT_d[:])
            bias_sb = cpool.tile([H, 7], f32)
            nc.sync.dma_start(bias_sb[:], bias_d[:])
            gsrc_sb = cpool.tile([128, NCHUNK], i32)
            nc.sync.dma_start(gsrc_sb[:], gsrc_d[:])
            scat_sb = cpool.tile([128, NCHUNK], i32)
            nc.sync.dma_start(scat_sb[:], scat_d[:])
            S_sb = cpool.tile([128, NCHUNK * 128], bf16)
            for c in range(NCHUNK):
                nc.sync.dma_start(S_sb[:, c * 128:(c + 1) * 128], S_d[c])
            SP_sb = cpool.tile([128, NT_LOC * N_GRAPHS], bf16)
            for t in range(NT_LOC):
                nc.sync.dma_start(SP_sb[:, t * N_GRAPHS:(t + 1) * N_GRAPHS], SP_d[t])
            zeros = cpool.tile([128, 640], f32)
            nc.vector.memset(zeros[:], 0.0)

            # ---- DRAM scratch ----
            nf_loc = dpool.tile([N_LOC, H], bf16)
            ebc_d = dpool.tile([32, 128, E_PAD], bf16)
            agg_part = dpool.tile([AGG_ROWS, H], f32)
            agg_loc = dpool.tile([N_LOC, H], f32)
            pool_part = dpool.tile([N_GRAPHS, H], f32)
            pool_out = dpool.tile([N_GRAPHS, H], f32, addr_space="Shared")

            # ---- persistent state ----
            EH1 = spool.tile([H, E_PAD], bf16)
            NF2t = [spool.tile([128, TW_MAX], bf16, name=f"NF2t{i}")
                    for i in range(len(TILES))]
            gbuf = spool.tile([128, NCHUNK * H], bf16)
            msg_rows = spool.tile([128, NCHUNK * H], bf16)
            scat_rows = spool.tile([128, NCHUNK * H], f32)
            nfrow = spool.tile([128, NT_LOC * H], bf16)
            hidA = spool.tile([H, N_PADLOC], f32)
            hidB = spool.tile([H, N_PADLOC], f32)
            xT = spool.tile([H, N_PADLOC], f32)
            nc.vector.memset(xT[:], 0.0)

            # ========== prep phase (transient inputs) ==========
            with tc.tile_pool(name="prep", bufs=1) as ppool:
                eft_sb = ppool.tile([EDGE_IN + 1, E_PAD], f32)
                nc.sync.dma_start(eft_sb[:], eft_d[:])
                nftl_sb = ppool.tile([NODE_IN, N_PADLOC], f32)
                nc.sync.dma_start(nftl_sb[:], nft_d[:])
                # edge hidden (once)
                for ti in range(E_PAD // 512):
                    sl = slice(ti * 512, (ti + 1) * 512)
                    ps = psT.tile([H, 512], f32, tag="t")
                    nc.tensor.matmul(ps[:], lhsT=W1a_sb[:], rhs=eft_sb[:, sl],
                                     start=True, stop=True)
                    nc.scalar.activation(EH1[:, sl], ps[:], ACT_RELU,
                                         bias=bias_sb[:, 6:7])
                # EBC: c-pair broadcasts of e_hid rows, step-invariant -> DRAM
                eh_dram = dpool.tile([H, E_PAD], bf16)
                nc.sync.dma_start(eh_dram[:], EH1[:])
                for j in range(32):
                    nc.sync.dma_start(
                        ebc_d[j, 0:64, :],
                        eh_dram[2 * j:2 * j + 1, :].to_broadcast((64, E_PAD)))
                    nc.sync.dma_start(
                        ebc_d[j, 64:128, :],
                        eh_dram[2 * j + 1:2 * j + 2, :].to_broadcast((64, E_PAD)))
                # initial node state (once)
                for t in range(NT_LOC):
                    sl = slice(t * 128, (t + 1) * 128)
                    ps = psT.tile([H, 128], f32, tag="t")
                    nc.tensor.matmul(ps[:], lhsT=Wp_sb[:], rhs=nftl_sb[:, sl],
                                     start=True, stop=True)
                    nc.scalar.activation(hidA[:, sl], ps[:], ACT_RELU,
                                         bias=bias_sb[:, 5:6])

            wstack = tc.tile_pool(name="work", bufs=2)
            wpool = wstack.__enter__()
            ebstack = tc.tile_pool(name="eb", bufs=3)
            ebpool = ebstack.__enter__()
            zstack = tc.tile_pool(name="zt", bufs=3)
            zpool = zstack.__enter__()

            def write_state_rows(hid):
                """hid^T [64, N_PADLOC] -> nfrow row tiles -> nf_loc (DRAM)."""
                for t in range(NT_LOC):
                    pt = psT.tile([128, H], f32, tag="t")
                    nc.tensor.transpose(pt[:], in_=hid[:, t * 128:(t + 1) * 128],
                                        identity=ident[:H, :H])
                    nc.scalar.activation(nfrow[:, t * H:(t + 1) * H], pt[:], ACT_COPY)
                for t in range(NT_LOC):
                    cnt = min(128, N_LOC - t * 128)
                    nc.sync.dma_start(nf_loc[t * 128:t * 128 + cnt, :],
                                      nfrow[:cnt, t * H:(t + 1) * H])

            write_state_rows(hidA)

            hid_cur, hid_nxt = hidA, hidB
            for step in range(STEPS):
                # ---- 1. zero the partial agg table (overlaps compute) ----
                for a in range(AGG_ROWS // 1280):
                    nc.sync.dma_start(
                        agg_part[a * 1280:(a + 1) * 1280, :].rearrange(
                            "(p r) h -> p (r h)", p=128),
                        zeros[:])

                # ---- 2. gather nf[src] rows (LOCAL) + transpose into NF2 ----
                for c in range(NCHUNK):
                    nc.gpsimd.indirect_dma_start(
                        out=gbuf[:, c * H:(c + 1) * H],
                        out_offset=None,
                        in_=nf_loc[:, :],
                        in_offset=bass.IndirectOffsetOnAxis(
                            ap=gsrc_sb[:, c:c + 1], axis=0))
                for ti, (t0, tw) in enumerate(TILES):
                    nf2 = NF2t[ti]
                    for cb in range(tw // 128):
                        c = t0 // 128 + cb
                        pt = psT.tile([H, 128], bf16, tag="t")
                        nc.tensor.transpose(pt[:], in_=gbuf[:, c * H:(c + 1) * H],
                                            identity=ident16[:, :])
                        nc.scalar.activation(nf2[0:H, cb * 128:(cb + 1) * 128],
                                             pt[:], ACT_COPY)
                    nc.sync.dma_start(nf2[H:128, :tw], nf2[0:H, :tw])

                # ---- 3. msg^T = W2P-chunks @ Z^T, wide tiles ----
                for ti, (t0, tw) in enumerate(TILES):
                    nf2 = NF2t[ti]
                    pm = psM.tile([H, TW_MAX], f32, tag="m")
                    for j in range(32):
                        eb = ebpool.tile([128, TW_MAX], bf16, tag="eb")
                        nc.sync.dma_start(eb[:, :tw], ebc_d[j, :, t0:t0 + tw])
                        zt = zpool.tile([128, TW_MAX], bf16, tag="zt")
                        nc.vector.tensor_mul(zt[:, :tw], eb[:, :tw], nf2[:, :tw])
                        for k in range(tw // 512):
                            ksl = slice(k * 512, (k + 1) * 512)
                            nc.tensor.matmul(pm[:, ksl],
                                             lhsT=W2P_sb[:, j * H:(j + 1) * H],
                                             rhs=zt[:, ksl], start=(j == 0),
                                             stop=False)
                    for k in range(tw // 512):
                        ksl = slice(k * 512, (k + 1) * 512)
                        nc.tensor.matmul(pm[:, ksl], lhsT=B2_sb[:],
                                         rhs=nf2[0:H, ksl],
                                         start=False, stop=True)
                    # transpose msg^T tile back to row layout (via SBUF)
                    mT = wpool.tile([H, TW_MAX], f32, tag="mT")
                    nc.scalar.activation(mT[:, :tw], pm[:, :tw], ACT_COPY)
                    for h4 in range(tw // 128):
                        cb = t0 // 128 + h4
                        pt = psT.tile([128, H], f32, tag="t")
                        nc.tensor.transpose(
                            pt[:], in_=mT[:, h4 * 128:(h4 + 1) * 128],
                            identity=ident[:H, :H])
                        nc.scalar.activation(
                            msg_rows[:, cb * H:(cb + 1) * H], pt[:], ACT_COPY)
                    # dedup + scatter for this tile's chunks
                    for h4 in range(tw // 128):
                        c = t0 // 128 + h4
                        ps = psT.tile([128, H], f32, tag="t")
                        nc.tensor.matmul(
                            ps[:], lhsT=S_sb[:, c * 128:(c + 1) * 128],
                            rhs=msg_rows[:, c * H:(c + 1) * H], start=True,
                            stop=True)
                        nc.scalar.activation(scat_rows[:, c * H:(c + 1) * H],
                                             ps[:], ACT_COPY)
                        nc.gpsimd.indirect_dma_start(
                            out=agg_part[:, :],
                            out_offset=bass.IndirectOffsetOnAxis(
                                ap=scat_sb[:, c:c + 1], axis=0),
                            in_=scat_rows[:, c * H:(c + 1) * H],
                            in_offset=None)

                # ---- 4. ReduceScatter partial aggs ----
                nc.gpsimd.collective_compute(
                    "ReduceScatter", mybir.AluOpType.add, replica_groups=RG,
                    ins=[agg_part[:N_NODES, :]], outs=[agg_loc[:]])

                # ---- 5. x^T = relu(agg + conv_bias) ----
                for t in range(NT_LOC):
                    cnt = min(128, N_LOC - t * 128)
                    xr = wpool.tile([128, H], f32, tag="xrow")
                    nc.sync.dma_start(xr[:cnt, :], agg_loc[t * 128:t * 128 + cnt, :])
                    pt = psT.tile([H, 128], f32, tag="t")
                    nc.tensor.transpose(pt[:, :cnt], in_=xr[:cnt, :],
                                        identity=ident[:cnt, :cnt])
                    nc.scalar.activation(xT[:, t * 128:t * 128 + cnt],
                                         pt[:, :cnt], ACT_RELU,
                                         bias=bias_sb[:, 4:5])

                # ---- 6. GRU (local nodes, fp32) ----
                for (n0, nw) in GRU_TILES:
                    xsl = xT[:, n0:n0 + nw]
                    hsl = hid_cur[:, n0:n0 + nw]
                    # r gate
                    pr = psG.tile([H, 512], f32, tag="gate")
                    nc.tensor.matmul(pr[:, :nw], lhsT=Wih_sb[:, 0:H], rhs=xsl,
                                     start=True, stop=False)
                    nc.tensor.matmul(pr[:, :nw], lhsT=Whh_sb[:, 0:H], rhs=hsl,
                                     start=False, stop=True)
                    r = wpool.tile([H, 512], f32, tag="gr", bufs=1)
                    nc.scalar.activation(r[:, :nw], pr[:, :nw], ACT_SIG,
                                         bias=bias_sb[:, 0:1])
                    # z gate
                    pz = psG.tile([H, 512], f32, tag="gate")
                    nc.tensor.matmul(pz[:, :nw], lhsT=Wih_sb[:, H:2 * H], rhs=xsl,
                                     start=True, stop=False)
                    nc.tensor.matmul(pz[:, :nw], lhsT=Whh_sb[:, H:2 * H], rhs=hsl,
                                     start=False, stop=True)
                    z = wpool.tile([H, 512], f32, tag="gz", bufs=1)
                    nc.scalar.activation(z[:, :nw], pz[:, :nw], ACT_SIG,
                                         bias=bias_sb[:, 1:2])
                    # n gate: n = tanh(i_n + b_in + r*(h_n + b_hn))
                    phn = psG.tile([H, 512], f32, tag="gate")
                    nc.tensor.matmul(phn[:, :nw], lhsT=Whh_sb[:, 2 * H:3 * H],
                                     rhs=hsl, start=True, stop=True)
                    hn = wpool.tile([H, 512], f32, tag="ghn", bufs=1)
                    nc.scalar.activation(hn[:, :nw], phn[:, :nw],
                                         mybir.ActivationFunctionType.Identity,
                                         bias=bias_sb[:, 3:4])
                    pin = psG.tile([H, 512], f32, tag="gate")
                    nc.tensor.matmul(pin[:, :nw], lhsT=Wih_sb[:, 2 * H:3 * H],
                                     rhs=xsl, start=True, stop=True)
                    rn = wpool.tile([H, 512], f32, tag="grn", bufs=1)
                    nc.vector.tensor_mul(rn[:, :nw], r[:, :nw], hn[:, :nw])
                    tmp = wpool.tile([H, 512], f32, tag="gtmp", bufs=1)
                    nc.vector.tensor_add(tmp[:, :nw], rn[:, :nw], pin[:, :nw])
                    n_g = wpool.tile([H, 512], f32, tag="gn", bufs=1)
                    nc.scalar.activation(n_g[:, :nw], tmp[:, :nw], ACT_TANH,
                                         bias=bias_sb[:, 2:3])
                    # h' = n + z*(h - n)
                    hmn = wpool.tile([H, 512], f32, tag="ghmn", bufs=1)
                    nc.vector.tensor_sub(hmn[:, :nw], hsl, n_g[:, :nw])
                    zh = wpool.tile([H, 512], f32, tag="gzh", bufs=1)
                    nc.vector.tensor_mul(zh[:, :nw], z[:, :nw], hmn[:, :nw])
                    nc.vector.tensor_add(hid_nxt[:, n0:n0 + nw], n_g[:, :nw],
                                         zh[:, :nw])

                # ---- 7. write new state rows (skip DMA on last step) ----
                if step < STEPS - 1:
                    write_state_rows(hid_nxt)
                else:
                    for t in range(NT_LOC):
                        pt = psT.tile([128, H], f32, tag="t")
                        nc.tensor.transpose(
                            pt[:], in_=hid_nxt[:, t * 128:(t + 1) * 128],
                            identity=ident[:H, :H])
                        nc.scalar.activation(nfrow[:, t * H:(t + 1) * H], pt[:],
                                             ACT_COPY)
                hid_cur, hid_nxt = hid_nxt, hid_cur

            # ========== sum pooling + AllReduce ==========
            pp = psM.tile([N_GRAPHS, H], f32, tag="m")
            for t in range(NT_LOC):
                nc.tensor.matmul(pp[:], lhsT=SP_sb[:, t * N_GRAPHS:(t + 1) * N_GRAPHS],
                                 rhs=nfrow[:, t * H:(t + 1) * H],
                                 start=(t == 0), stop=(t == NT_LOC - 1))
            pool_sb = wpool.tile([N_GRAPHS, H], f32, tag="pool")
            nc.scalar.activation(pool_sb[:], pp[:], ACT_COPY)
            nc.sync.dma_start(pool_part[:], pool_sb[:])
            nc.gpsimd.collective_compute(
                "AllReduce", mybir.AluOpType.add, replica_groups=RG,
                ins=[pool_part[:]], outs=[pool_out[:]])
            nc.sync.dma_start(out_d[:], pool_out[:])
            zstack.__exit__(None, None, None)
            ebstack.__exit__(None, None, None)
            wstack.__exit__(None, None, None)

    nc.compile()
    return nc


_CACHED = {}


def _get_nc():
    if "nc" not in _CACHED:
        _CACHED["nc"] = build_bass()
    return _CACHED["nc"]


def kernel(**inputs):
    np_inputs = {k: np.asarray(v) for k, v in inputs.items()}
    in_maps = prep_host(np_inputs)
    nc = _get_nc()
    res = bass_utils.run_bass_kernel_spmd(
        nc, in_maps, core_ids=list(range(NCORES)))
    return res.results[0]["g_feat"]


if __name__ == "__main__":
    print("kernel module OK")


# revision 14
# speedup vs baseline: 1.6738x; 1.0735x over previous
"""MPNN (LocalRetro) message-passing kernel for 8 Trainium2 NeuronCores.

Strategy (SPMD, one program, per-core input data):
- Edges sharded by SRC-node range: core i owns edges whose src lies in its
  1250-node slice, dst-sorted within the core, packed into 128-slot chunks
  so that no dst group spans a chunk boundary. The nf[src] gather is then
  core-LOCAL (no AllGather). Pad slots yield e_hid == 0 via a -1e9
  pad-indicator row appended to the edge-network input.
- msg = (e_hid (x) nf[src]) @ W2' computed without materializing edge_W:
  the Khatri-Rao product Z^T is built on DVE in fp16 as EBC_j * NF2 where
  EBC_j (the j-th c-pair of e_hid rows broadcast across partitions) is
  step-invariant and precomputed once into DRAM via broadcast DMA, then
  streamed per step. Contraction on the PE against host-permuted W2 chunks
  (32 x K=128) in wide 2048-column tiles.
- Aggregation: a per-chunk one-hot matmul collapses each chunk's edges into
  per-dst-node rows (exact fp32); indirect DMA places the rows into the
  core's partial agg table over ALL nodes (rows never collide within a
  core). A ReduceScatter sums partials across cores; each core runs the
  GRU for its 1250 nodes in fp32.
- Final sum-pooling via one-hot matmuls + AllReduce; core 0's output is
  returned.
"""
import sys

sys.path.insert(0, "/opt/trn_rl_repo")

import numpy as np
import ml_dtypes

from concourse import bass, bacc, mybir, tile, bass_utils
from concourse.masks import make_identity

BF16 = np.float16  # fp16: 10 mantissa bits, same PE/DVE rates as bf16

NCORES = 8
N_NODES = 10000
N_EDGES = 50000
N_GRAPHS = 128
NODE_IN = 74
EDGE_IN = 12
H = 64
STEPS = 6

N_LOC = N_NODES // NCORES            # 1250
N_PADLOC = 1280                      # local nodes padded to 10 tiles
NT_LOC = N_PADLOC // 128             # 10
NCHUNK = 52                          # 128-slot edge chunks per core
E_PAD = NCHUNK * 128                 # 6656 edge slots
TILES = [(0, 2048), (2048, 2048), (4096, 2048), (6144, 512)]
TW_MAX = 2048
GBATCH = 1                           # chunks per indirect-DMA call
TRASH = N_NODES                      # agg rows >= N_NODES are scratch
AGG_ROWS = 10240                     # zeroed region (>= TRASH + 128)
GRU_TILES = [(0, 512), (512, 512), (1024, 256)]
ACT_RELU = mybir.ActivationFunctionType.Relu
ACT_SIG = mybir.ActivationFunctionType.Sigmoid
ACT_TANH = mybir.ActivationFunctionType.Tanh
ACT_COPY = mybir.ActivationFunctionType.Copy


def prep_host(inp):
    """Build per-core and shared device input arrays from the full inputs."""
    W_e2 = inp["W_e2"]
    # W2 chunks for the c-pair Khatri-Rao layout:
    # chunk j, row i      -> (c=2j,   h=i)
    # chunk j, row 64+i   -> (c=2j+1, h=i)
    W2r = W_e2.reshape(H, H, H)
    W2P = np.zeros((32, 128, H), np.float32)
    i = np.arange(64)
    for j in range(32):
        W2P[j, :64, :] = W2r[2 * j, i, :]
        W2P[j, 64:, :] = W2r[2 * j + 1, i, :]
    b_ih, b_hh = inp["b_ih"], inp["b_hh"]
    b_rz = b_ih + b_hh
    bias_cols = np.stack([
        b_rz[0:64], b_rz[64:128],          # r, z sigmoid biases
        b_ih[128:192], b_hh[128:192],      # n-gate: i_n bias, h_n bias
        inp["conv_bias"], inp["b_proj"], inp["b_e1"],
    ], axis=1).astype(np.float32)          # [64, 7]

    shared = {
        "W_proj": inp["W_proj"].astype(np.float32),
        "W1a": np.concatenate([inp["W_e1"],
                               np.full((1, H), -1e9, np.float32)], 0),
        "W2P": W2P.astype(BF16),
        "B2": inp["b_e2"].reshape(H, H).astype(BF16),
        "WihT": np.ascontiguousarray(inp["W_ih"].T).astype(np.float32),
        "WhhT": np.ascontiguousarray(inp["W_hh"].T).astype(np.float32),
        "bias_cols": bias_cols,
    }

    src, dst = np.asarray(inp["src"]), np.asarray(inp["dst"])
    ef, gids = np.asarray(inp["edge_feats"]), np.asarray(inp["graph_ids"])
    nft = np.ascontiguousarray(np.asarray(inp["node_feats"]).T).astype(np.float32)
    order = np.argsort(dst, kind="stable")
    dst_s, src_s, ef_s = dst[order], src[order], ef[order]

    cores = []
    for ci in range(NCORES):
        lo, hi = ci * N_LOC, (ci + 1) * N_LOC
        sel = (src_s >= lo) & (src_s < hi)       # shard by SRC range
        d, s, e = dst_s[sel], src_s[sel], ef_s[sel]
        ne = len(d)
        assert ne <= E_PAD, f"core {ci}: {ne} edges > {E_PAD}"
        starts = np.flatnonzero(np.concatenate([[True], d[1:] != d[:-1]])) if ne else np.array([], np.int64)
        ends = np.concatenate([starts[1:], [ne]]) if ne else np.array([], np.int64)
        slot_src = np.zeros(E_PAD, np.int32)     # LOCAL src row index
        slot_pad = np.ones(E_PAD, np.float32)
        slot_ef = np.zeros((E_PAD, EDGE_IN), np.float32)
        S = np.zeros((NCHUNK, 128, 128), np.float32)
        scat_idx = np.tile(np.arange(TRASH, TRASH + 128, dtype=np.int32)[:, None],
                           (1, NCHUNK))
        chunk, pos, slot = 0, 0, 0
        for g in range(len(starts)):
            glen = int(ends[g] - starts[g])
            assert glen <= 128, "node degree exceeds one chunk"
            if pos + glen > 128:
                chunk, pos, slot = chunk + 1, 0, 0
            assert chunk < NCHUNK, "NCHUNK too small"
            b = chunk * 128
            sl = slice(int(starts[g]), int(ends[g]))
            slot_src[b + pos:b + pos + glen] = s[sl] - lo
            slot_pad[b + pos:b + pos + glen] = 0.0
            slot_ef[b + pos:b + pos + glen] = e[sl]
            S[chunk, pos:pos + glen, slot] = 1.0
            scat_idx[slot, chunk] = int(d[int(starts[g])])
            pos += glen
            slot += 1
        eft = np.zeros((EDGE_IN + 1, E_PAD), np.float32)
        eft[:EDGE_IN] = slot_ef.T
        eft[EDGE_IN] = slot_pad
        g_loc = gids[lo:hi]
        SP = np.zeros((NT_LOC, 128, N_GRAPHS), np.float32)
        for t in range(NT_LOC):
            cnt = min(128, N_LOC - t * 128)
            SP[t, np.arange(cnt), g_loc[t * 128:t * 128 + cnt]] = 1.0
        nft_loc = np.zeros((NODE_IN, N_PADLOC), np.float32)
        nft_loc[:, :N_LOC] = nft[:, lo:hi]
        core = dict(shared)
        core.update({
            "eft": eft,
            "gsrc": np.ascontiguousarray(slot_src.reshape(NCHUNK, 128).T).astype(np.int32),
            "scat_idx": scat_idx.astype(np.int32),
            "S": S.astype(BF16), "SP": SP.astype(BF16), "nft_loc": nft_loc,
        })
        cores.append(core)
    return cores


def build_bass():
    nc = bacc.Bacc("TRN2", target_bir_lowering=False, debug=False,
                   num_devices=NCORES)
    dt = mybir.dt
    f32, bf16, i32 = dt.float32, dt.float16, dt.int32
    f8 = dt.float8e4

    def din(name, shape, dtype):
        return nc.dram_tensor(name, shape, dtype, kind="ExternalInput")

    nft_d = din("nft_loc", [NODE_IN, N_PADLOC], f32)
    Wp_d = din("W_proj", [NODE_IN, H], f32)
    W1a_d = din("W1a", [EDGE_IN + 1, H], f32)
    W2P_d = din("W2P", [32, 128, H], bf16)
    B2_d = din("B2", [H, H], bf16)
    WihT_d = din("WihT", [H, 3 * H], f32)
    WhhT_d = din("WhhT", [H, 3 * H], f32)
    bias_d = din("bias_cols", [H, 7], f32)
    eft_d = din("eft", [EDGE_IN + 1, E_PAD], f32)
    gsrc_d = din("gsrc", [128, NCHUNK], i32)
    scat_d = din("scat_idx", [128, NCHUNK], i32)
    S_d = din("S", [NCHUNK, 128, 128], bf16)
    SP_d = din("SP", [NT_LOC, 128, N_GRAPHS], bf16)
    out_d = nc.dram_tensor("g_feat", [N_GRAPHS, H], f32, kind="ExternalOutput")

    RG = [list(range(NCORES))]

    with tile.TileContext(nc) as tc:
        with tc.tile_pool(name="const", bufs=1) as cpool, \
             tc.tile_pool(name="state", bufs=1) as spool, \
             tc.tile_pool(name="psM", bufs=1, space="PSUM") as psM, \
             tc.tile_pool(name="psT", bufs=2, space="PSUM") as psT, \
             tc.tile_pool(name="psG", bufs=2, space="PSUM") as psG, \
             tc.tile_pool(name="dram", bufs=1, space="DRAM") as dpool:

            ident = cpool.tile([128, 128], f32)
            make_identity(nc, ident[:])
            ident16 = cpool.tile([128, 128], bf16)
            make_identity(nc, ident16[:])

            # ---- constants to SBUF ----
            Wp_sb = cpool.tile([NODE_IN, H], f32)
            nc.sync.dma_start(Wp_sb[:], Wp_d[:])
            W1a_sb = cpool.tile([EDGE_IN + 1, H], f32)
            nc.sync.dma_start(W1a_sb[:], W1a_d[:])
            W2P_sb = cpool.tile([128, 32 * H], bf16)
            for j in range(32):
                nc.sync.dma_start(W2P_sb[:, j * H:(j + 1) * H], W2P_d[j])
            B2_sb = cpool.tile([H, H], bf16)
            nc.sync.dma_start(B2_sb[:], B2_d[:])
            Wih_sb = cpool.tile([H, 3 * H], f32)
            nc.sync.dma_start(Wih_sb[:], WihT_d[:])
            Whh_sb = cpool.tile([H, 3 * H], f32)
            nc.sync.dma_start(Whh_sb[:], WhhT_d[:])
            bias_sb = cpool.tile([H, 7], f32)
            nc.sync.dma_start(bias_sb[:], bias_d[:])
            gsrc_sb = cpool.tile([128, NCHUNK], i32)
            nc.sync.dma_start(gsrc_sb[:], gsrc_d[:])
            scat_sb = cpool.tile([128, NCHUNK], i32)
            nc.sync.dma_start(scat_sb[:], scat_d[:])
            S_sb = cpool.tile([128, NCHUNK * 128], bf16)
            for c in range(NCHUNK):
                nc.sync.dma_start(S_sb[:, c * 128:(c + 1) * 128], S_d[c])
            SP_sb = cpool.tile([128, NT_LOC * N_GRAPHS], bf16)
            for t in range(NT_LOC):
                nc.sync.dma_start(SP_sb[:, t * N_GRAPHS:(t + 1) * N_GRAPHS], SP_d[t])
            zeros = cpool.tile([128, 640], f32)
            nc.vector.memset(zeros[:], 0.0)

            # ---- DRAM scratch ----
            nf_loc = dpool.tile([N_LOC, H], bf16)
            ebc_d = dpool.tile([32, 128, E_PAD], bf16)
            agg_part = dpool.tile([AGG_ROWS, H], f32)
            agg_loc = dpool.tile([N_LOC, H], f32)
            pool_part = dpool.tile([N_GRAPHS, H], f32)
            pool_out = dpool.tile([N_GRAPHS, H], f32, addr_space="Shared")

            # ---- persistent state ----
            EH1 = spool.tile([H, E_PAD], bf16)
            NF2t = [spool.tile([128, TW_MAX], bf16, name=f"NF2t{i}")
                    for i in range(len(TILES))]
            gbuf = spool.tile([128, NCHUNK * H], bf16)
            msg_rows = spool.tile([128, NCHUNK * H], bf16)
            scat_rows = spool.tile([128, NCHUNK * H], f32)
            nfrow = spool.tile([128, NT_LOC * H], bf16)
            hidA = spool.tile([H, N_PADLOC], f32)
            hidB = spool.tile([H, N_PADLOC], f32)
            xT = spool.tile([H, N_PADLOC], f32)
            nc.vector.memset(xT[:], 0.0)

            # ========== prep phase (transient inputs) ==========
            with tc.tile_pool(name="prep", bufs=1) as ppool:
                eft_sb = ppool.tile([EDGE_IN + 1, E_PAD], f32)
                nc.sync.dma_start(eft_sb[:], eft_d[:])
                nftl_sb = ppool.tile([NODE_IN, N_PADLOC], f32)
                nc.sync.dma_start(nftl_sb[:], nft_d[:])
                # edge hidden (once)
                for ti in range(E_PAD // 512):
                    sl = slice(ti * 512, (ti + 1) * 512)
                    ps = psT.tile([H, 512], f32, tag="t")
                    nc.tensor.matmul(ps[:], lhsT=W1a_sb[:], rhs=eft_sb[:, sl],
                                     start=True, stop=True)
                    nc.scalar.activation(EH1[:, sl], ps[:], ACT_RELU,
                                         bias=bias_sb[:, 6:7])
                # EBC: c-pair broadcasts of e_hid rows, step-invariant -> DRAM
                eh_dram = dpool.tile([H, E_PAD], bf16)
                nc.sync.dma_start(eh_dram[:], EH1[:])
                for j in range(32):
                    nc.sync.dma_start(
                        ebc_d[j, 0:64, :],
                        eh_dram[2 * j:2 * j + 1, :].to_broadcast((64, E_PAD)))
                    nc.sync.dma_start(
                        ebc_d[j, 64:128, :],
                        eh_dram[2 * j + 1:2 * j + 2, :].to_broadcast((64, E_PAD)))
                # initial node state (once)
                for t in range(NT_LOC):
                    sl = slice(t * 128, (t + 1) * 128)
                    ps = psT.tile([H, 128], f32, tag="t")
                    nc.tensor.matmul(ps[:], lhsT=Wp_sb[:], rhs=nftl_sb[:, sl],
                                     start=True, stop=True)
                    nc.scalar.activation(hidA[:, sl], ps[:], ACT_RELU,
                                         bias=bias_sb[:, 5:6])

            wstack = tc.tile_pool(name="work", bufs=2)
            wpool = wstack.__enter__()
            ebstack = tc.tile_pool(name="eb", bufs=3)
            ebpool = ebstack.__enter__()
            zstack = tc.tile_pool(name="zt", bufs=3)
            zpool = zstack.__enter__()

            def write_state_rows(hid):
                """hid^T [64, N_PADLOC] -> nfrow row tiles -> nf_loc (DRAM)."""
                for t in range(NT_LOC):
                    pt = psT.tile([128, H], f32, tag="t")
                    nc.tensor.transpose(pt[:], in_=hid[:, t * 128:(t + 1) * 128],
                                        identity=ident[:H, :H])
                    nc.scalar.activation(nfrow[:, t * H:(t + 1) * H], pt[:], ACT_COPY)
                for t in range(NT_LOC):
                    cnt = min(128, N_LOC - t * 128)
                    nc.sync.dma_start(nf_loc[t * 128:t * 128 + cnt, :],
                                      nfrow[:cnt, t * H:(t + 1) * H])

            write_state_rows(hidA)

            hid_cur, hid_nxt = hidA, hidB
            for step in range(STEPS):
                # ---- 1. zero the partial agg table (overlaps compute) ----
                for a in range(AGG_ROWS // 1280):
                    nc.sync.dma_start(
                        agg_part[a * 1280:(a + 1) * 1280, :].rearrange(
                            "(p r) h -> p (r h)", p=128),
                        zeros[:])

                # ---- 2. gather nf[src] rows (LOCAL) + transpose into NF2 ----
                for c in range(0, NCHUNK, GBATCH):
                    nc.gpsimd.indirect_dma_start(
                        out=gbuf[:, c * H:(c + GBATCH) * H],
                        out_offset=None,
                        in_=nf_loc[:, :],
                        in_offset=bass.IndirectOffsetOnAxis(
                            ap=gsrc_sb[:, c:c + GBATCH], axis=0))
                for ti, (t0, tw) in enumerate(TILES):
                    nf2 = NF2t[ti]
                    for cb in range(tw // 128):
                        c = t0 // 128 + cb
                        pt = psT.tile([H, 128], bf16, tag="t")
                        nc.tensor.transpose(pt[:], in_=gbuf[:, c * H:(c + 1) * H],
                                            identity=ident16[:, :])
                        nc.scalar.activation(nf2[0:H, cb * 128:(cb + 1) * 128],
                                             pt[:], ACT_COPY)
                    nc.sync.dma_start(nf2[H:128, :tw], nf2[0:H, :tw])

                # ---- 3. msg^T = W2P-chunks @ Z^T, wide tiles ----
                for ti, (t0, tw) in enumerate(TILES):
                    nf2 = NF2t[ti]
                    pm = psM.tile([H, TW_MAX], f32, tag="m")
                    for j in range(32):
                        eb = ebpool.tile([128, TW_MAX], bf16, tag="eb")
                        nc.sync.dma_start(eb[:, :tw], ebc_d[j, :, t0:t0 + tw])
                        zt = zpool.tile([128, TW_MAX], bf16, tag="zt")
                        nc.vector.tensor_mul(zt[:, :tw], eb[:, :tw], nf2[:, :tw])
                        for k in range(tw // 512):
                            ksl = slice(k * 512, (k + 1) * 512)
                            nc.tensor.matmul(pm[:, ksl],
                                             lhsT=W2P_sb[:, j * H:(j + 1) * H],
                                             rhs=zt[:, ksl], start=(j == 0),
                                             stop=False)
                    for k in range(tw // 512):
                        ksl = slice(k * 512, (k + 1) * 512)
                        nc.tensor.matmul(pm[:, ksl], lhsT=B2_sb[:],
                                         rhs=nf2[0:H, ksl],
                                         start=False, stop=True)
                    # transpose msg^T tile back to row layout (via SBUF)
                    mT = wpool.tile([H, TW_MAX], f32, tag="mT")
                    nc.scalar.activation(mT[:, :tw], pm[:, :tw], ACT_COPY)
                    for h4 in range(tw // 128):
                        cb = t0 // 128 + h4
                        pt = psT.tile([128, H], f32, tag="t")
                        nc.tensor.transpose(
                            pt[:], in_=mT[:, h4 * 128:(h4 + 1) * 128],
                            identity=ident[:H, :H])
                        nc.scalar.activation(
                            msg_rows[:, cb * H:(cb + 1) * H], pt[:], ACT_COPY)
                    # dedup + batched scatter for this tile's chunks
                    for h4 in range(tw // 128):
                        c = t0 // 128 + h4
                        ps = psT.tile([128, H], f32, tag="t")
                        nc.tensor.matmul(
                            ps[:], lhsT=S_sb[:, c * 128:(c + 1) * 128],
                            rhs=msg_rows[:, c * H:(c + 1) * H], start=True,
                            stop=True)
                        nc.scalar.activation(scat_rows[:, c * H:(c + 1) * H],
                                             ps[:], ACT_COPY)
                        if (c + 1) % GBATCH == 0:
                            cb0 = c + 1 - GBATCH
                            nc.gpsimd.indirect_dma_start(
                                out=agg_part[:, :],
                                out_offset=bass.IndirectOffsetOnAxis(
                                    ap=scat_sb[:, cb0:cb0 + GBATCH], axis=0),
                                in_=scat_rows[:, cb0 * H:(cb0 + GBATCH) * H],
                                in_offset=None)

                # ---- 4. ReduceScatter partial aggs ----
                nc.gpsimd.collective_compute(
                    "ReduceScatter", mybir.AluOpType.add, replica_groups=RG,
                    ins=[agg_part[:N_NODES, :]], outs=[agg_loc[:]])

                # ---- 5. x^T = relu(agg + conv_bias) ----
                for t in range(NT_LOC):
                    cnt = min(128, N_LOC - t * 128)
                    xr = wpool.tile([128, H], f32, tag="xrow")
                    nc.sync.dma_start(xr[:cnt, :], agg_loc[t * 128:t * 128 + cnt, :])
                    pt = psT.tile([H, 128], f32, tag="t")
                    nc.tensor.transpose(pt[:, :cnt], in_=xr[:cnt, :],
                                        identity=ident[:cnt, :cnt])
                    nc.scalar.activation(xT[:, t * 128:t * 128 + cnt],
                                         pt[:, :cnt], ACT_RELU,
                                         bias=bias_sb[:, 4:5])

                # ---- 6. GRU (local nodes, fp32) ----
                for (n0, nw) in GRU_TILES:
                    xsl = xT[:, n0:n0 + nw]
                    hsl = hid_cur[:, n0:n0 + nw]
                    # r gate
                    pr = psG.tile([H, 512], f32, tag="gate")
                    nc.tensor.matmul(pr[:, :nw], lhsT=Wih_sb[:, 0:H], rhs=xsl,
                                     start=True, stop=False)
                    nc.tensor.matmul(pr[:, :nw], lhsT=Whh_sb[:, 0:H], rhs=hsl,
                                     start=False, stop=True)
                    r = wpool.tile([H, 512], f32, tag="gr", bufs=1)
                    nc.scalar.activation(r[:, :nw], pr[:, :nw], ACT_SIG,
                                         bias=bias_sb[:, 0:1])
                    # z gate
                    pz = psG.tile([H, 512], f32, tag="gate")
                    nc.tensor.matmul(pz[:, :nw], lhsT=Wih_sb[:, H:2 * H], rhs=xsl,
                                     start=True, stop=False)
                    nc.tensor.matmul(pz[:, :nw], lhsT=Whh_sb[:, H:2 * H], rhs=hsl,
                                     start=False, stop=True)
                    z = wpool.tile([H, 512], f32, tag="gz", bufs=1)
                    nc.scalar.activation(z[:, :nw], pz[:, :nw], ACT_SIG,
                                         bias=bias_sb[:, 1:2])
                    # n gate: n = tanh(i_n + b_in + r*(h_n + b_hn))
                    phn = psG.tile([H, 512], f32, tag="gate")
                    nc.tensor.matmul(phn[:, :nw], lhsT=Whh_sb[:, 2 * H:3 * H],
                                     rhs=hsl, start=True, stop=True)
                    hn = wpool.tile([H, 512], f32, tag="ghn", bufs=1)
                    nc.scalar.activation(hn[:, :nw], phn[:, :nw],
                                         mybir.ActivationFunctionType.Identity,
                                         bias=bias_sb[:, 3:4])
                    pin = psG.tile([H, 512], f32, tag="gate")
                    nc.tensor.matmul(pin[:, :nw], lhsT=Wih_sb[:, 2 * H:3 * H],
                                     rhs=xsl, start=True, stop=True)
                    rn = wpool.tile([H, 512], f32, tag="grn", bufs=1)
                    nc.vector.tensor_mul(rn[:, :nw], r[:, :nw], hn[:, :nw])
                    tmp = wpool.tile([H, 512], f32, tag="gtmp", bufs=1)
                    nc.vector.tensor_add(tmp[:, :nw], rn[:, :nw], pin[:, :nw])
                    n_g = wpool.tile([H, 512], f32, tag="gn", bufs=1)
                    nc.scalar.activation(n_g[:, :nw], tmp[:, :nw], ACT_TANH,
                                         bias=bias_sb[:, 2:3])
                    # h' = n + z*(h - n)
                    hmn = wpool.tile([H, 512], f32, tag="ghmn", bufs=1)
                    nc.vector.tensor_sub(hmn[:, :nw], hsl, n_g[:, :nw])
                    zh = wpool.tile([H, 512], f32, tag="gzh", bufs=1)
                    nc.vector.tensor_mul(zh[:, :nw], z[:, :nw], hmn[:, :nw])
                    nc.vector.tensor_add(hid_nxt[:, n0:n0 + nw], n_g[:, :nw],
                                         zh[:, :nw])

                # ---- 7. write new state rows (skip DMA on last step) ----
                if step < STEPS - 1:
                    write_state_rows(hid_nxt)
                else:
                    for t in range(NT_LOC):
                        pt = psT.tile([128, H], f32, tag="t")
                        nc.tensor.transpose(
                            pt[:], in_=hid_nxt[:, t * 128:(t + 1) * 128],
                            identity=ident[:H, :H])
                        nc.scalar.activation(nfrow[:, t * H:(t + 1) * H], pt[:],
                                             ACT_COPY)
                hid_cur, hid_nxt = hid_nxt, hid_cur

            # ========== sum pooling + AllReduce ==========
            pp = psM.tile([N_GRAPHS, H], f32, tag="m")
            for t in range(NT_LOC):
                nc.tensor.matmul(pp[:], lhsT=SP_sb[:, t * N_GRAPHS:(t + 1) * N_GRAPHS],
                                 rhs=nfrow[:, t * H:(t + 1) * H],
                                 start=(t == 0), stop=(t == NT_LOC - 1))
            pool_sb = wpool.tile([N_GRAPHS, H], f32, tag="pool")
            nc.scalar.activation(pool_sb[:], pp[:], ACT_COPY)
            nc.sync.dma_start(pool_part[:], pool_sb[:])
            nc.gpsimd.collective_compute(
                "AllReduce", mybir.AluOpType.add, replica_groups=RG,
                ins=[pool_part[:]], outs=[pool_out[:]])
            nc.sync.dma_start(out_d[:], pool_out[:])
            zstack.__exit__(None, None, None)
            ebstack.__exit__(None, None, None)
            wstack.__exit__(None, None, None)

    nc.compile()
    return nc


_CACHED = {}


def _get_nc():
    if "nc" not in _CACHED:
        _CACHED["nc"] = build_bass()
    return _CACHED["nc"]


def kernel(**inputs):
    np_inputs = {k: np.asarray(v) for k, v in inputs.items()}
    in_maps = prep_host(np_inputs)
    nc = _get_nc()
    res = bass_utils.run_bass_kernel_spmd(
        nc, in_maps, core_ids=list(range(NCORES)))
    return res.results[0]["g_feat"]


if __name__ == "__main__":
    print("kernel module OK")


# revision 17
# speedup vs baseline: 1.6788x; 1.0030x over previous
"""MPNN (LocalRetro) message-passing kernel for 8 Trainium2 NeuronCores.

Strategy (SPMD, one program, per-core input data):
- Edges sharded by SRC-node range: core i owns edges whose src lies in its
  1250-node slice, dst-sorted within the core, packed into 128-slot chunks
  so that no dst group spans a chunk boundary. The nf[src] gather is then
  core-LOCAL (no AllGather). Pad slots yield e_hid == 0 via a -1e9
  pad-indicator row appended to the edge-network input.
- msg = (e_hid (x) nf[src]) @ W2' computed without materializing edge_W:
  the Khatri-Rao product Z^T is built on DVE in fp16 as EBC_j * NF2 where
  EBC_j (the j-th c-pair of e_hid rows broadcast across partitions) is
  step-invariant and precomputed once into DRAM via broadcast DMA, then
  streamed per step. Contraction on the PE against host-permuted W2 chunks
  (32 x K=128) in wide 2048-column tiles.
- Aggregation: a per-chunk one-hot matmul collapses each chunk's edges into
  per-dst-node rows (exact fp32); indirect DMA places the rows into the
  core's partial agg table over ALL nodes (rows never collide within a
  core). A ReduceScatter sums partials across cores; each core runs the
  GRU for its 1250 nodes in fp32.
- Final sum-pooling via one-hot matmuls + AllReduce; core 0's output is
  returned.
"""
import sys

sys.path.insert(0, "/opt/trn_rl_repo")

import numpy as np
import ml_dtypes

from concourse import bass, bacc, mybir, tile, bass_utils
from concourse.masks import make_identity

BF16 = np.float16  # fp16: 10 mantissa bits, same PE/DVE rates as bf16

NCORES = 8
N_NODES = 10000
N_EDGES = 50000
N_GRAPHS = 128
NODE_IN = 74
EDGE_IN = 12
H = 64
STEPS = 6

N_LOC = N_NODES // NCORES            # 1250
N_PADLOC = 1280                      # local nodes padded to 10 tiles
NT_LOC = N_PADLOC // 128             # 10
NCHUNK = 52                          # 128-slot edge chunks per core
E_PAD = NCHUNK * 128                 # 6656 edge slots
TILES = [(0, 2048), (2048, 2048), (4096, 2048), (6144, 512)]
TW_MAX = 2048
GBATCH = 1                           # chunks per indirect-DMA call
TRASH = N_NODES                      # agg rows >= N_NODES are scratch
AGG_ROWS = 10240                     # zeroed region (>= TRASH + 128)
GRU_TILES = [(0, 512), (512, 512), (1024, 256)]
ACT_RELU = mybir.ActivationFunctionType.Relu
ACT_SIG = mybir.ActivationFunctionType.Sigmoid
ACT_TANH = mybir.ActivationFunctionType.Tanh
ACT_COPY = mybir.ActivationFunctionType.Copy


def prep_host(inp):
    """Build per-core and shared device input arrays from the full inputs."""
    W_e2 = inp["W_e2"]
    # W2 chunks for the c-pair Khatri-Rao layout:
    # chunk j, row i      -> (c=2j,   h=i)
    # chunk j, row 64+i   -> (c=2j+1, h=i)
    W2r = W_e2.reshape(H, H, H)
    W2P = np.zeros((32, 128, H), np.float32)
    i = np.arange(64)
    for j in range(32):
        W2P[j, :64, :] = W2r[2 * j, i, :]
        W2P[j, 64:, :] = W2r[2 * j + 1, i, :]
    b_ih, b_hh = inp["b_ih"], inp["b_hh"]
    b_rz = b_ih + b_hh
    bias_cols = np.stack([
        b_rz[0:64], b_rz[64:128],          # r, z sigmoid biases
        b_ih[128:192], b_hh[128:192],      # n-gate: i_n bias, h_n bias
        inp["conv_bias"], inp["b_proj"], inp["b_e1"],
    ], axis=1).astype(np.float32)          # [64, 7]

    shared = {
        "W_proj": inp["W_proj"].astype(np.float32),
        "W1a": np.concatenate([inp["W_e1"],
                               np.full((1, H), -1e9, np.float32)], 0),
        "W2P": W2P.astype(BF16),
        "B2": inp["b_e2"].reshape(H, H).astype(BF16),
        "WihT": np.ascontiguousarray(inp["W_ih"].T).astype(np.float32),
        "WhhT": np.ascontiguousarray(inp["W_hh"].T).astype(np.float32),
        "bias_cols": bias_cols,
    }

    src, dst = np.asarray(inp["src"]), np.asarray(inp["dst"])
    ef, gids = np.asarray(inp["edge_feats"]), np.asarray(inp["graph_ids"])
    nft = np.ascontiguousarray(np.asarray(inp["node_feats"]).T).astype(np.float32)
    order = np.argsort(dst, kind="stable")
    dst_s, src_s, ef_s = dst[order], src[order], ef[order]

    cores = []
    for ci in range(NCORES):
        lo, hi = ci * N_LOC, (ci + 1) * N_LOC
        sel = (src_s >= lo) & (src_s < hi)       # shard by SRC range
        d, s, e = dst_s[sel], src_s[sel], ef_s[sel]
        ne = len(d)
        assert ne <= E_PAD, f"core {ci}: {ne} edges > {E_PAD}"
        starts = np.flatnonzero(np.concatenate([[True], d[1:] != d[:-1]])) if ne else np.array([], np.int64)
        ends = np.concatenate([starts[1:], [ne]]) if ne else np.array([], np.int64)
        slot_src = np.zeros(E_PAD, np.int32)     # LOCAL src row index
        slot_pad = np.ones(E_PAD, np.float32)
        slot_ef = np.zeros((E_PAD, EDGE_IN), np.float32)
        S = np.zeros((NCHUNK, 128, 128), np.float32)
        scat_idx = np.tile(np.arange(TRASH, TRASH + 128, dtype=np.int32)[:, None],
                           (1, NCHUNK))
        chunk, pos, slot = 0, 0, 0
        for g in range(len(starts)):
            glen = int(ends[g] - starts[g])
            assert glen <= 128, "node degree exceeds one chunk"
            if pos + glen > 128:
                chunk, pos, slot = chunk + 1, 0, 0
            assert chunk < NCHUNK, "NCHUNK too small"
            b = chunk * 128
            sl = slice(int(starts[g]), int(ends[g]))
            slot_src[b + pos:b + pos + glen] = s[sl] - lo
            slot_pad[b + pos:b + pos + glen] = 0.0
            slot_ef[b + pos:b + pos + glen] = e[sl]
            S[chunk, pos:pos + glen, slot] = 1.0
            scat_idx[slot, chunk] = int(d[int(starts[g])])
            pos += glen
            slot += 1
        eft = np.zeros((EDGE_IN + 1, E_PAD), np.float32)
        eft[:EDGE_IN] = slot_ef.T
        eft[EDGE_IN] = slot_pad
        g_loc = gids[lo:hi]
        SP = np.zeros((NT_LOC, 128, N_GRAPHS), np.float32)
        for t in range(NT_LOC):
            cnt = min(128, N_LOC - t * 128)
            SP[t, np.arange(cnt), g_loc[t * 128:t * 128 + cnt]] = 1.0
        nft_loc = np.zeros((NODE_IN, N_PADLOC), np.float32)
        nft_loc[:, :N_LOC] = nft[:, lo:hi]
        core = dict(shared)
        core.update({
            "eft": eft,
            "gsrc": np.ascontiguousarray(slot_src.reshape(NCHUNK, 128).T).astype(np.int32),
            "scat_idx": scat_idx.astype(np.int32),
            "S": S.astype(BF16), "SP": SP.astype(BF16), "nft_loc": nft_loc,
        })
        cores.append(core)
    return cores


def build_bass():
    nc = bacc.Bacc("TRN2", target_bir_lowering=False, debug=False,
                   num_devices=NCORES)
    dt = mybir.dt
    f32, bf16, i32 = dt.float32, dt.float16, dt.int32
    f8 = dt.float8e4

    def din(name, shape, dtype):
        return nc.dram_tensor(name, shape, dtype, kind="ExternalInput")

    nft_d = din("nft_loc", [NODE_IN, N_PADLOC], f32)
    Wp_d = din("W_proj", [NODE_IN, H], f32)
    W1a_d = din("W1a", [EDGE_IN + 1, H], f32)
    W2P_d = din("W2P", [32, 128, H], bf16)
    B2_d = din("B2", [H, H], bf16)
    WihT_d = din("WihT", [H, 3 * H], f32)
    WhhT_d = din("WhhT", [H, 3 * H], f32)
    bias_d = din("bias_cols", [H, 7], f32)
    eft_d = din("eft", [EDGE_IN + 1, E_PAD], f32)
    gsrc_d = din("gsrc", [128, NCHUNK], i32)
    scat_d = din("scat_idx", [128, NCHUNK], i32)
    S_d = din("S", [NCHUNK, 128, 128], bf16)
    SP_d = din("SP", [NT_LOC, 128, N_GRAPHS], bf16)
    out_d = nc.dram_tensor("g_feat", [N_GRAPHS, H], f32, kind="ExternalOutput")

    RG = [list(range(NCORES))]

    with tile.TileContext(nc) as tc:
        with tc.tile_pool(name="const", bufs=1) as cpool, \
             tc.tile_pool(name="state", bufs=1) as spool, \
             tc.tile_pool(name="psM", bufs=1, space="PSUM") as psM, \
             tc.tile_pool(name="psT", bufs=2, space="PSUM") as psT, \
             tc.tile_pool(name="psG", bufs=2, space="PSUM") as psG, \
             tc.tile_pool(name="dram", bufs=1, space="DRAM") as dpool:

            ident = cpool.tile([128, 128], f32)
            make_identity(nc, ident[:])
            ident16 = cpool.tile([128, 128], bf16)
            make_identity(nc, ident16[:])

            # ---- constants to SBUF ----
            Wp_sb = cpool.tile([NODE_IN, H], f32)
            nc.sync.dma_start(Wp_sb[:], Wp_d[:])
            W1a_sb = cpool.tile([EDGE_IN + 1, H], f32)
            nc.sync.dma_start(W1a_sb[:], W1a_d[:])
            W2P_sb = cpool.tile([128, 32 * H], bf16)
            for j in range(32):
                nc.sync.dma_start(W2P_sb[:, j * H:(j + 1) * H], W2P_d[j])
            B2_sb = cpool.tile([H, H], bf16)
            nc.sync.dma_start(B2_sb[:], B2_d[:])
            Wih_sb = cpool.tile([H, 3 * H], f32)
            nc.sync.dma_start(Wih_sb[:], WihT_d[:])
            Whh_sb = cpool.tile([H, 3 * H], f32)
            nc.sync.dma_start(Whh_sb[:], WhhT_d[:])
            bias_sb = cpool.tile([H, 7], f32)
            nc.sync.dma_start(bias_sb[:], bias_d[:])
            gsrc_sb = cpool.tile([128, NCHUNK], i32)
            nc.sync.dma_start(gsrc_sb[:], gsrc_d[:])
            scat_sb = cpool.tile([128, NCHUNK], i32)
            nc.sync.dma_start(scat_sb[:], scat_d[:])
            S_sb = cpool.tile([128, NCHUNK * 128], bf16)
            for c in range(NCHUNK):
                nc.sync.dma_start(S_sb[:, c * 128:(c + 1) * 128], S_d[c])
            SP_sb = cpool.tile([128, NT_LOC * N_GRAPHS], bf16)
            for t in range(NT_LOC):
                nc.sync.dma_start(SP_sb[:, t * N_GRAPHS:(t + 1) * N_GRAPHS], SP_d[t])
            zeros = cpool.tile([128, 640], f32)
            nc.vector.memset(zeros[:], 0.0)

            # ---- DRAM scratch ----
            nf_loc = dpool.tile([N_LOC, H], bf16)
            ebc_ts = [dpool.tile([32, 128, tw], bf16, name=f"ebc{ti}")
                      for ti, (t0, tw) in enumerate(TILES)]
            agg_part = dpool.tile([AGG_ROWS, H], f32)
            agg_loc = dpool.tile([N_LOC, H], f32)
            pool_part = dpool.tile([N_GRAPHS, H], f32)
            pool_out = dpool.tile([N_GRAPHS, H], f32, addr_space="Shared")

            # ---- persistent state ----
            EH1 = spool.tile([H, E_PAD], bf16)
            NF2t = [spool.tile([128, TW_MAX], bf16, name=f"NF2t{i}")
                    for i in range(len(TILES))]
            gbuf = spool.tile([128, NCHUNK * H], bf16)
            msg_rows = spool.tile([128, NCHUNK * H], bf16)
            scat_rows = spool.tile([128, NCHUNK * H], f32)
            nfrow = spool.tile([128, NT_LOC * H], bf16)
            hidA = spool.tile([H, N_PADLOC], f32)
            hidB = spool.tile([H, N_PADLOC], f32)
            xT = spool.tile([H, N_PADLOC], f32)
            nc.vector.memset(xT[:], 0.0)

            # ========== prep phase (transient inputs) ==========
            with tc.tile_pool(name="prep", bufs=1) as ppool:
                eft_sb = ppool.tile([EDGE_IN + 1, E_PAD], f32)
                nc.sync.dma_start(eft_sb[:], eft_d[:])
                nftl_sb = ppool.tile([NODE_IN, N_PADLOC], f32)
                nc.sync.dma_start(nftl_sb[:], nft_d[:])
                # edge hidden (once)
                for ti in range(E_PAD // 512):
                    sl = slice(ti * 512, (ti + 1) * 512)
                    ps = psT.tile([H, 512], f32, tag="t")
                    nc.tensor.matmul(ps[:], lhsT=W1a_sb[:], rhs=eft_sb[:, sl],
                                     start=True, stop=True)
                    nc.scalar.activation(EH1[:, sl], ps[:], ACT_RELU,
                                         bias=bias_sb[:, 6:7])
                # EBC: c-pair broadcasts of e_hid rows, step-invariant -> DRAM
                eh_dram = dpool.tile([H, E_PAD], bf16)
                nc.sync.dma_start(eh_dram[:], EH1[:])
                # initial node state (once)
                for t in range(NT_LOC):
                    sl = slice(t * 128, (t + 1) * 128)
                    ps = psT.tile([H, 128], f32, tag="t")
                    nc.tensor.matmul(ps[:], lhsT=Wp_sb[:], rhs=nftl_sb[:, sl],
                                     start=True, stop=True)
                    nc.scalar.activation(hidA[:, sl], ps[:], ACT_RELU,
                                         bias=bias_sb[:, 5:6])

            wstack = tc.tile_pool(name="work", bufs=2)
            wpool = wstack.__enter__()
            ebstack = tc.tile_pool(name="eb", bufs=3)
            ebpool = ebstack.__enter__()
            zstack = tc.tile_pool(name="zt", bufs=3)
            zpool = zstack.__enter__()

            def write_state_rows(hid):
                """hid^T [64, N_PADLOC] -> nfrow row tiles -> nf_loc (DRAM)."""
                for t in range(NT_LOC):
                    pt = psT.tile([128, H], f32, tag="t")
                    nc.tensor.transpose(pt[:], in_=hid[:, t * 128:(t + 1) * 128],
                                        identity=ident[:H, :H])
                    nc.scalar.activation(nfrow[:, t * H:(t + 1) * H], pt[:], ACT_COPY)
                for t in range(NT_LOC):
                    cnt = min(128, N_LOC - t * 128)
                    nc.sync.dma_start(nf_loc[t * 128:t * 128 + cnt, :],
                                      nfrow[:cnt, t * H:(t + 1) * H])

            write_state_rows(hidA)

            # EBC build: after the state write so step-1 gathers aren't
            # head-of-line blocked; tile-major so step-1 reads pipeline.
            for ti, (t0, tw) in enumerate(TILES):
                for j in range(32):
                    nc.sync.dma_start(
                        ebc_ts[ti][j, 0:64, :],
                        eh_dram[2 * j:2 * j + 1,
                                t0:t0 + tw].to_broadcast((64, tw)))
                    nc.sync.dma_start(
                        ebc_ts[ti][j, 64:128, :],
                        eh_dram[2 * j + 1:2 * j + 2,
                                t0:t0 + tw].to_broadcast((64, tw)))

            hid_cur, hid_nxt = hidA, hidB
            for step in range(STEPS):
                # ---- 1. zero the partial agg table (overlaps compute) ----
                for a in range(AGG_ROWS // 1280):
                    nc.sync.dma_start(
                        agg_part[a * 1280:(a + 1) * 1280, :].rearrange(
                            "(p r) h -> p (r h)", p=128),
                        zeros[:])

                # ---- 2. gather nf[src] rows (LOCAL) + transpose into NF2 ----
                for c in range(0, NCHUNK, GBATCH):
                    nc.gpsimd.indirect_dma_start(
                        out=gbuf[:, c * H:(c + GBATCH) * H],
                        out_offset=None,
                        in_=nf_loc[:, :],
                        in_offset=bass.IndirectOffsetOnAxis(
                            ap=gsrc_sb[:, c:c + GBATCH], axis=0))
                for ti, (t0, tw) in enumerate(TILES):
                    nf2 = NF2t[ti]
                    for cb in range(tw // 128):
                        c = t0 // 128 + cb
                        pt = psT.tile([H, 128], bf16, tag="t")
                        nc.tensor.transpose(pt[:], in_=gbuf[:, c * H:(c + 1) * H],
                                            identity=ident16[:, :])
                        nc.scalar.activation(nf2[0:H, cb * 128:(cb + 1) * 128],
                                             pt[:], ACT_COPY)
                    nc.sync.dma_start(nf2[H:128, :tw], nf2[0:H, :tw])

                # ---- 3. msg^T = W2P-chunks @ Z^T, wide tiles ----
                for ti, (t0, tw) in enumerate(TILES):
                    nf2 = NF2t[ti]
                    pm = psM.tile([H, TW_MAX], f32, tag="m")
                    for j in range(32):
                        eb = ebpool.tile([128, TW_MAX], bf16, tag="eb")
                        nc.sync.dma_start(eb[:, :tw], ebc_ts[ti][j, :, :])
                        zt = zpool.tile([128, TW_MAX], bf16, tag="zt")
                        nc.vector.tensor_mul(zt[:, :tw], eb[:, :tw], nf2[:, :tw])
                        for k in range(tw // 512):
                            ksl = slice(k * 512, (k + 1) * 512)
                            nc.tensor.matmul(pm[:, ksl],
                                             lhsT=W2P_sb[:, j * H:(j + 1) * H],
                                             rhs=zt[:, ksl], start=(j == 0),
                                             stop=False)
                    for k in range(tw // 512):
                        ksl = slice(k * 512, (k + 1) * 512)
                        nc.tensor.matmul(pm[:, ksl], lhsT=B2_sb[:],
                                         rhs=nf2[0:H, ksl],
                                         start=False, stop=True)
                    # transpose msg^T tile back to row layout (via SBUF)
                    mT = wpool.tile([H, TW_MAX], f32, tag="mT")
                    nc.scalar.activation(mT[:, :tw], pm[:, :tw], ACT_COPY)
                    for h4 in range(tw // 128):
                        cb = t0 // 128 + h4
                        pt = psT.tile([128, H], f32, tag="t")
                        nc.tensor.transpose(
                            pt[:], in_=mT[:, h4 * 128:(h4 + 1) * 128],
                            identity=ident[:H, :H])
                        nc.scalar.activation(
                            msg_rows[:, cb * H:(cb + 1) * H], pt[:], ACT_COPY)
                    # dedup + batched scatter for this tile's chunks
                    for h4 in range(tw // 128):
                        c = t0 // 128 + h4
                        ps = psT.tile([128, H], f32, tag="t")
                        nc.tensor.matmul(
                            ps[:], lhsT=S_sb[:, c * 128:(c + 1) * 128],
                            rhs=msg_rows[:, c * H:(c + 1) * H], start=True,
                            stop=True)
                        nc.scalar.activation(scat_rows[:, c * H:(c + 1) * H],
                                             ps[:], ACT_COPY)
                        nc.gpsimd.indirect_dma_start(
                            out=agg_part[:, :],
                            out_offset=bass.IndirectOffsetOnAxis(
                                ap=scat_sb[:, c:c + 1], axis=0),
                            in_=scat_rows[:, c * H:(c + 1) * H],
                            in_offset=None)

                # ---- 4. ReduceScatter partial aggs ----
                nc.gpsimd.collective_compute(
                    "ReduceScatter", mybir.AluOpType.add, replica_groups=RG,
                    ins=[agg_part[:N_NODES, :]], outs=[agg_loc[:]])

                # ---- 5. x^T = relu(agg + conv_bias), phased with the GRU --
                def build_xt(trange):
                    for t in trange:
                        cnt = min(128, N_LOC - t * 128)
                        xr = wpool.tile([128, H], f32, tag="xrow")
                        nc.sync.dma_start(xr[:cnt, :],
                                          agg_loc[t * 128:t * 128 + cnt, :])
                        pt = psT.tile([H, 128], f32, tag="t")
                        nc.tensor.transpose(pt[:, :cnt], in_=xr[:cnt, :],
                                            identity=ident[:cnt, :cnt])
                        nc.scalar.activation(xT[:, t * 128:t * 128 + cnt],
                                             pt[:, :cnt], ACT_RELU,
                                             bias=bias_sb[:, 4:5])

                build_xt(range(0, NT_LOC))

                # ---- 6. GRU (local nodes, fp32) ----
                for gi_, (n0, nw) in enumerate(GRU_TILES):
                    xsl = xT[:, n0:n0 + nw]
                    hsl = hid_cur[:, n0:n0 + nw]
                    # r gate
                    pr = psG.tile([H, 512], f32, tag="gate")
                    nc.tensor.matmul(pr[:, :nw], lhsT=Wih_sb[:, 0:H], rhs=xsl,
                                     start=True, stop=False)
                    nc.tensor.matmul(pr[:, :nw], lhsT=Whh_sb[:, 0:H], rhs=hsl,
                                     start=False, stop=True)
                    r = wpool.tile([H, 512], f32, tag="gr", bufs=1)
                    nc.scalar.activation(r[:, :nw], pr[:, :nw], ACT_SIG,
                                         bias=bias_sb[:, 0:1])
                    # z gate
                    pz = psG.tile([H, 512], f32, tag="gate")
                    nc.tensor.matmul(pz[:, :nw], lhsT=Wih_sb[:, H:2 * H], rhs=xsl,
                                     start=True, stop=False)
                    nc.tensor.matmul(pz[:, :nw], lhsT=Whh_sb[:, H:2 * H], rhs=hsl,
                                     start=False, stop=True)
                    z = wpool.tile([H, 512], f32, tag="gz", bufs=1)
                    nc.scalar.activation(z[:, :nw], pz[:, :nw], ACT_SIG,
                                         bias=bias_sb[:, 1:2])
                    # n gate: n = tanh(i_n + b_in + r*(h_n + b_hn))
                    phn = psG.tile([H, 512], f32, tag="gate")
                    nc.tensor.matmul(phn[:, :nw], lhsT=Whh_sb[:, 2 * H:3 * H],
                                     rhs=hsl, start=True, stop=True)
                    hn = wpool.tile([H, 512], f32, tag="ghn", bufs=1)
                    nc.scalar.activation(hn[:, :nw], phn[:, :nw],
                                         mybir.ActivationFunctionType.Identity,
                                         bias=bias_sb[:, 3:4])
                    pin = psG.tile([H, 512], f32, tag="gate")
                    nc.tensor.matmul(pin[:, :nw], lhsT=Wih_sb[:, 2 * H:3 * H],
                                     rhs=xsl, start=True, stop=True)
                    rn = wpool.tile([H, 512], f32, tag="grn", bufs=1)
                    nc.vector.tensor_mul(rn[:, :nw], r[:, :nw], hn[:, :nw])
                    tmp = wpool.tile([H, 512], f32, tag="gtmp", bufs=1)
                    nc.vector.tensor_add(tmp[:, :nw], rn[:, :nw], pin[:, :nw])
                    n_g = wpool.tile([H, 512], f32, tag="gn", bufs=1)
                    nc.scalar.activation(n_g[:, :nw], tmp[:, :nw], ACT_TANH,
                                         bias=bias_sb[:, 2:3])
                    # h' = n + z*(h - n)
                    hmn = wpool.tile([H, 512], f32, tag="ghmn", bufs=1)
                    nc.vector.tensor_sub(hmn[:, :nw], hsl, n_g[:, :nw])
                    zh = wpool.tile([H, 512], f32, tag="gzh", bufs=1)
                    nc.vector.tensor_mul(zh[:, :nw], z[:, :nw], hmn[:, :nw])
                    nc.vector.tensor_add(hid_nxt[:, n0:n0 + nw], n_g[:, :nw],
                                         zh[:, :nw])

                # ---- 7. write new state rows (skip DMA on last step) ----
                if step < STEPS - 1:
                    write_state_rows(hid_nxt)
                else:
                    for t in range(NT_LOC):
                        pt = psT.tile([128, H], f32, tag="t")
                        nc.tensor.transpose(
                            pt[:], in_=hid_nxt[:, t * 128:(t + 1) * 128],
                            identity=ident[:H, :H])
                        nc.scalar.activation(nfrow[:, t * H:(t + 1) * H], pt[:],
                                             ACT_COPY)
                hid_cur, hid_nxt = hid_nxt, hid_cur

            # ========== sum pooling + AllReduce ==========
            pp = psM.tile([N_GRAPHS, H], f32, tag="m")
            for t in range(NT_LOC):
                nc.tensor.matmul(pp[:], lhsT=SP_sb[:, t * N_GRAPHS:(t + 1) * N_GRAPHS],
                                 rhs=nfrow[:, t * H:(t + 1) * H],
                                 start=(t == 0), stop=(t == NT_LOC - 1))
            pool_sb = wpool.tile([N_GRAPHS, H], f32, tag="pool")
            nc.scalar.activation(pool_sb[:], pp[:], ACT_COPY)
            nc.sync.dma_start(pool_part[:], pool_sb[:])
            nc.gpsimd.collective_compute(
                "AllReduce", mybir.AluOpType.add, replica_groups=RG,
                ins=[pool_part[:]], outs=[pool_out[:]])
            nc.sync.dma_start(out_d[:], pool_out[:])
            zstack.__exit__(None, None, None)
            ebstack.__exit__(None, None, None)
            wstack.__exit__(None, None, None)

    nc.compile()
    return nc


_CACHED = {}


def _get_nc():
    if "nc" not in _CACHED:
        _CACHED["nc"] = build_bass()
    return _CACHED["nc"]


def kernel(**inputs):
    np_inputs = {k: np.asarray(v) for k, v in inputs.items()}
    in_maps = prep_host(np_inputs)
    nc = _get_nc()
    res = bass_utils.run_bass_kernel_spmd(
        nc, in_maps, core_ids=list(range(NCORES)))
    return res.results[0]["g_feat"]


if __name__ == "__main__":
    print("kernel module OK")
